# revision 50
# baseline (speedup 1.0000x reference)
"""Trainium2 Bass kernel for nn_Detector (batched FPS detector head).

Pipeline per core (256 submaps = 2 tiles of 128 submaps x 1024 points):
  1. Load pos interleaved (split across DMA queues), split channels.
  2. v = px^2 + py^2 per point (exact f32, matches reference ranking).
  3. Exact 512-smallest threshold per submap via bit-space binary search:
     15 rounds on the u16 high halves of v (4x-packed DVE compares), then
     16 rounds on full f32 v with thresholds composed bitwise from u16
     halves (exact; avoids f32 integer rounding above 2^24).
  4. Stream-compact the 512 selected points per submap (cumsum ranks +
     per-partition local_scatter of coords as u16 halves). Negated coords
     + global x-row index go to DRAM so the FPS loop can fetch its pick's
     center (as ready-made Square biases) via one tiny indirect DMA.
  5. 32-point farthest-point sampling, two tiles software-pipelined:
     d^2 terms are single ACT Square(p + (-c)) ops (exact FMA + square),
     sums/min on Pool/DVE, argmax on DVE. Per pick, the x feature row is
     gathered immediately so the MLP runs in chunks during the loop.
  6. Tiny MLP on the 32 selected rows (block-diagonal weights, 4 lanes),
     relu via exact tensor_scalar max; softplus(z) = ln(1 + exp(z)) with
     per-chunk Exp during the loop and one fused Ln in the tail (exactly
     two activation-table loads in the whole program).

Outputs per core: weights [256, 32] f32, indices [256, 32] int32.
"""

import sys

for _p in ("/opt/trn_rl_repo",):
    if _p not in sys.path:
        sys.path.insert(0, _p)

import numpy as np

import concourse.bass as bass
import concourse.bacc as bacc
import concourse.mybir as mybir
from concourse.bass import IndirectOffsetOnAxis
from concourse.mybir import ActivationFunctionType as actf
from concourse.mybir import AluOpType as alu
from concourse.tile import TileContext

f32 = mybir.dt.float32
i32 = mybir.dt.int32
i16 = mybir.dt.int16
u16 = mybir.dt.uint16
u32 = mybir.dt.uint32

P = 128          # partitions = submaps per tile
NPTS = 1024      # points per submap
KPOS = 512       # closest points kept
K = 32           # FPS samples per submap
TILES = 2        # tiles per core
S_CORE = P * TILES  # submaps per core
N_CORES = 8


def build_nc():
    nc = bacc.Bacc()

    pos_in = nc.declare_dram_parameter("pos", [S_CORE * NPTS, 3], f32, isOutput=False)
    x_in = nc.declare_dram_parameter("x", [S_CORE * NPTS, 32], f32, isOutput=False)
    w1d_in = nc.declare_dram_parameter("W1d", [128, 64], f32, isOutput=False)
    w2d_in = nc.declare_dram_parameter("W2d", [64, 32], f32, isOutput=False)
    w3d_in = nc.declare_dram_parameter("W3d", [32, 4], f32, isOutput=False)
    b1d_in = nc.declare_dram_parameter("b1d", [64, 1], f32, isOutput=False)
    b2d_in = nc.declare_dram_parameter("b2d", [32, 1], f32, isOutput=False)
    b3d_in = nc.declare_dram_parameter("b3d", [4, 1], f32, isOutput=False)
    eye_in = nc.declare_dram_parameter("eye128", [128, 128], f32, isOutput=False)

    w_out = nc.declare_dram_parameter("weights_out", [S_CORE, K], f32, isOutput=True)
    i_out = nc.declare_dram_parameter("indices_out", [S_CORE, K], i32, isOutput=True)

    # DRAM scratch: compacted rows [-x, -y, -z, global_row_bits] per point
    pc_dram = [
        nc.dram_tensor(f"pc_dram{t}", [P * KPOS, 4], f32) for t in range(TILES)
    ]

    pos_t = pos_in[:].rearrange("(t p f) c -> t p (f c)", t=TILES, p=P)

    with TileContext(nc) as tc, tc.tile_pool(name="main", bufs=1) as pool:
        # ---- shared constants ----
        eye = pool.tile([P, 128], f32, tag="eye")
        nc.sync.dma_start(out=eye[:], in_=eye_in[:])
        rb512 = pool.tile([P, 1], i32, tag="rb512")
        nc.gpsimd.iota(rb512[:], [[1, 1]], base=0, channel_multiplier=KPOS)
        rb512f = pool.tile([P, 1], f32, tag="rb512f")
        nc.vector.tensor_copy(rb512f[:], rb512[:])
        iota16 = pool.tile([P, NPTS], i16, tag="iota16")
        nc.gpsimd.iota(iota16[:], [[1, NPTS]], channel_multiplier=0)
        io512i = pool.tile([P, KPOS], i32, tag="io512i")
        nc.gpsimd.iota(io512i[:], [[1, KPOS]], channel_multiplier=0)
        io512f = pool.tile([P, KPOS], f32, tag="io512f")
        nc.vector.tensor_copy(io512f[:], io512i[:])

        w1d = pool.tile([P, 64], f32, tag="w1d")
        nc.sync.dma_start(out=w1d[:], in_=w1d_in[:])
        w2d = pool.tile([64, 32], f32, tag="w2d")
        nc.sync.dma_start(out=w2d[:], in_=w2d_in[:])
        w3d = pool.tile([32, 4], f32, tag="w3d")
        nc.sync.dma_start(out=w3d[:], in_=w3d_in[:])
        b1d = pool.tile([64, 1], f32, tag="b1d")
        nc.sync.dma_start(out=b1d[:], in_=b1d_in[:])
        b2d = pool.tile([32, 1], f32, tag="b2d")
        nc.sync.dma_start(out=b2d[:], in_=b2d_in[:])
        b3d = pool.tile([4, 1], f32, tag="b3d")
        nc.sync.dma_start(out=b3d[:], in_=b3d_in[:])
        b3n = pool.tile([4, 1], f32, tag="b3n")
        nc.vector.tensor_scalar_mul(b3n[:], b3d[:], -1.0)
        cm1 = pool.tile([P, 1], f32, tag="cm1")
        nc.vector.memset(cm1[:], -1.0)
        cm2 = pool.tile([P, 1], f32, tag="cm2")
        nc.vector.memset(cm2[:], -2.0)

        # ---- load pos: three DMA thirds per tile, one per DGE engine ----
        T3 = NPTS
        pil, xrowf, xrowi = [], [], []
        for t in range(TILES):
            p_t = pool.tile([P, NPTS * 3], f32, tag=f"pil{t}", name=f"pil{t}")
            for j, eng in enumerate((nc.sync, nc.scalar, nc.gpsimd)):
                eng.dma_start(
                    out=p_t[:, j * T3 : (j + 1) * T3],
                    in_=pos_t[t][:, j * T3 : (j + 1) * T3],
                )
            pil.append(p_t)
            xr = pool.tile([P, 1], i32, tag=f"xrow{t}", name=f"xrow{t}")
            nc.gpsimd.iota(xr[:], [[1, 1]], base=t * P * NPTS, channel_multiplier=NPTS)
            xrowi.append(xr)
            xrf = pool.tile([P, 1], f32, tag=f"xrowf{t}", name=f"xrowf{t}")
            nc.vector.tensor_copy(xrf[:], xr[:])
            xrowf.append(xrf)

        # ---- split channels + v = px^2 + py^2; u16 high halves of v ----
        v, vh, pch = [], [], []
        for t in range(TILES):
            p3 = pil[t][:].rearrange("p (n c) -> p n c", c=3)
            chans = []
            for c, eng in ((0, nc.gpsimd), (1, nc.vector), (2, nc.scalar)):
                ch = pool.tile([P, NPTS], f32, tag=f"ch{c}_{t}", name=f"ch{c}_{t}")
                if eng is nc.scalar:
                    eng.copy(ch[:], p3[:, :, c])
                else:
                    eng.tensor_copy(ch[:], p3[:, :, c])
                chans.append(ch)
            pch.append(chans)
            sq = pool.tile([P, NPTS], f32, tag=f"sq{t}", name=f"sq{t}")
            nc.gpsimd.tensor_tensor(sq[:], chans[0][:], chans[0][:], alu.mult)
            sqy = pool.tile([P, NPTS], f32, tag=f"sqy{t}", name=f"sqy{t}")
            nc.vector.tensor_tensor(sqy[:], chans[1][:], chans[1][:], alu.mult)
            vt = pool.tile([P, NPTS], f32, tag=f"v{t}", name=f"v{t}")
            nc.gpsimd.tensor_tensor(vt[:], sq[:], sqy[:], alu.add)
            v.append(vt)
            vht = pool.tile([P, NPTS], u16, tag=f"vh{t}", name=f"vh{t}")
            v16 = vt[:].bitcast(u16).rearrange("p (n two) -> p n two", two=2)
            nc.scalar.copy(vht[:], v16[:, :, 1])
            vh.append(vht)

        # ---- bit-space binary search for the 512th-smallest v ----
        # T = bit pattern of the 512th smallest (positive floats are monotone
        # as unsigned ints). Build T bit by bit: lo += 2^i unless
        # count(bits <= lo + 2^i - 1) >= 512. Rounds 30..16 compare only the
        # u16 high halves (exact); rounds 15..0 compare full f32 v against a
        # threshold whose bits are composed from u16 halves (exact compose).
        bst = []
        for t in range(TILES):
            b = {}
            b["loH"] = pool.tile([P, 1], f32, tag=f"loH{t}", name=f"loH{t}")
            nc.vector.memset(b["loH"][:], 0.0)
            b["loL"] = pool.tile([P, 1], f32, tag=f"loL{t}", name=f"loL{t}")
            nc.vector.memset(b["loL"][:], 0.0)
            for nm in ("thrf", "cnt", "cond"):
                b[nm] = pool.tile([P, 1], f32, tag=f"{nm}{t}", name=f"{nm}{t}")
            b["thr"] = pool.tile([P, 1], f32, tag=f"thr{t}", name=f"thr{t}")
            b["junk16"] = pool.tile([P, NPTS], u16, tag=f"junk16_{t}", name=f"junk16_{t}")
            b["junkf"] = pool.tile([P, NPTS], f32, tag=f"sq{t}", name=f"junkf{t}")
            bst.append(b)

        # stage A: 15 rounds on u16 high halves (sign bit of v is 0).
        # Each tile runs its whole search back-to-back (tile 0 first) so
        # tile 0's Pool-heavy compaction overlaps tile 1's DVE-bound search.
        def stage_a(t):
            for i in range(14, -1, -1):
                b = bst[t]
                nc.vector.tensor_scalar(
                    b["thrf"][:], b["loH"][:], float(2**i - 1), None, alu.add
                )
                nc.vector.tensor_scalar(
                    b["junk16"][:], vh[t][:], b["thrf"][:], None,
                    alu.is_le, alu.add, accum_out=b["cnt"][:],
                )
                nc.vector.tensor_scalar(
                    b["cond"][:], b["cnt"][:], 512.0, None, alu.is_lt
                )
                nc.vector.tensor_scalar(
                    b["loH"][:], b["cond"][:], float(2**i), b["loH"][:],
                    alu.mult, alu.add,
                )
        # stage B operates on the u16 LOW halves, masked so that only
        # points whose high half equals H* participate: vlm = vl where
        # vh == loH else 0xFFFF. Probed thresholds never reach 0xFFFF
        # (thr = loL + 2^i - 1 <= 0xFFFF - 2^i), so masked points never
        # count. The target rank becomes 512 - count(vh < loH).
        def vlm_setup(t):
            b = bst[t]
            vlt = pool.tile([P, NPTS], u16, tag=f"junkf{t}x", name=f"vl{t}")
            v16 = v[t][:].bitcast(u16).rearrange("p (n two) -> p n two", two=2)
            nc.scalar.copy(vlt[:], v16[:, :, 0])
            eq = pool.tile([P, NPTS], u16, tag=f"mask{t}", name=f"eq{t}")
            nc.vector.tensor_scalar(eq[:], vh[t][:], b["loH"][:], None, alu.is_equal)
            t1 = pool.tile([P, NPTS], u16, tag=f"rank{t}", name=f"t1_{t}")
            nc.vector.tensor_tensor(t1[:], vlt[:], eq[:], alu.mult)
            t2 = pool.tile([P, NPTS], u16, tag=f"am{t}", name=f"t2_{t}")
            nc.vector.tensor_scalar(t2[:], eq[:], -65535.0, 65535.0, alu.mult, alu.add)
            b["vlm"] = pool.tile([P, NPTS], u16, tag=f"junkf{t}y", name=f"vlm{t}")
            nc.vector.tensor_tensor(b["vlm"][:], t1[:], t2[:], alu.add)
            # target rank among the vh == H* points
            nc.vector.tensor_scalar(
                b["junk16"][:], vh[t][:], b["loH"][:], None,
                alu.is_lt, alu.add, accum_out=b["cnt"][:],
            )
            b["tgt"] = pool.tile([P, 1], f32, tag=f"tgt{t}", name=f"tgt{t}")
            nc.gpsimd.tensor_scalar(
                b["tgt"][:], b["cnt"][:], -1.0, 512.0, alu.mult, alu.add
            )
        def stage_b(t):
            for i in range(15, -1, -1):
                b = bst[t]
                nc.vector.tensor_scalar(
                    b["thrf"][:], b["loL"][:], float(2**i - 1), None, alu.add
                )
                nc.vector.tensor_scalar(
                    b["junk16"][:], b["vlm"][:], b["thrf"][:], None,
                    alu.is_le, alu.add, accum_out=b["cnt"][:],
                )
                nc.vector.tensor_scalar(
                    b["cond"][:], b["cnt"][:], b["tgt"][:], None, alu.is_lt
                )
                nc.vector.tensor_scalar(
                    b["loL"][:], b["cond"][:], float(2**i), b["loL"][:],
                    alu.mult, alu.add,
                )
        def compose_tau(t):
            # tau = bitcast((loH << 16) | loL)
            b = bst[t]
            thr16 = b["thr"][:].bitcast(u16).rearrange("p (n two) -> p n two", two=2)
            nc.vector.tensor_copy(thr16[:, :, 1], b["loH"][:])
            nc.vector.tensor_copy(thr16[:, :, 0], b["loL"][:])

        # ---- compact: mask -> ranks -> scatter coords/indices ----
        pcc, grow32, grow_f = [None, None], [None, None], [None, None]
        sel16s = [None, None]

        def compact(t):
            b = bst[t]
            mask = pool.tile([P, NPTS], f32, tag=f"mask{t}", name=f"mask{t}")
            nc.vector.tensor_scalar(mask[:], v[t][:], b["thr"][:], None, alu.is_le)
            rank = pool.tile([P, NPTS], f32, tag=f"rank{t}", name=f"rank{t}")
            nc.vector.tensor_tensor_scan(
                rank[:], mask[:], mask[:], 0.0, alu.add, alu.bypass
            )
            am = pool.tile([P, NPTS], f32, tag=f"am{t}", name=f"am{t}")
            nc.vector.tensor_tensor(am[:], rank[:], mask[:], alu.mult)

            # u16-half slot indices: even half -> 2a-2, odd half -> 2a-1
            idx2 = pool.tile([P, 2 * NPTS], i16, tag=f"idx2_{t}", name=f"idx2_{t}")
            i2v = idx2[:].rearrange("p (n two) -> p n two", two=2)
            nc.scalar.activation(i2v[:, :, 0], am[:], actf.Identity, bias=cm2[:], scale=2.0)
            nc.scalar.activation(i2v[:, :, 1], am[:], actf.Identity, bias=cm1[:], scale=2.0)
            slot16 = pool.tile([P, NPTS], i16, tag=f"slot16{t}", name=f"slot16{t}")
            nc.scalar.activation(slot16[:], am[:], actf.Identity, bias=cm1[:], scale=1.0)

            # compacted coord channels (as u16 halves of f32)
            chc = []
            for c in range(3):
                cc = pool.tile([P, 2 * KPOS], u16, tag=f"cc{c}_{t}", name=f"cc{c}_{t}")
                nc.gpsimd.local_scatter(
                    cc[:], pch[t][c][:].bitcast(u16), idx2[:],
                    channels=P, num_elems=2 * KPOS, num_idxs=2 * NPTS,
                )
                chc.append(cc)
            pcc[t] = [cc[:].bitcast(f32) for cc in chc]

            # compacted original indices (i16 scatter of iota, then widen)
            sel16 = pool.tile([P, KPOS], i16, tag=f"sel16{t}", name=f"sel16{t}")
            nc.gpsimd.local_scatter(
                sel16[:], iota16[:], slot16[:],
                channels=P, num_elems=KPOS, num_idxs=NPTS,
            )
            sel16s[t] = sel16

        def compact_finish(t):
            # row-index conversion + DRAM scratch write: gates only the c4
            # gathers of the FPS loop, so it runs after both tiles' searches
            sel16 = sel16s[t]
            self_f = pool.tile([P, KPOS], f32, tag=f"selff{t}", name=f"selff{t}")
            nc.vector.tensor_copy(self_f[:], sel16[:])
            growf = pool.tile([P, KPOS], f32, tag=f"growf{t}", name=f"growf{t}")
            nc.vector.tensor_scalar(growf[:], self_f[:], xrowf[t][:], None, alu.add)
            g32 = pool.tile([P, KPOS], i32, tag=f"g32_{t}", name=f"g32_{t}")
            nc.vector.tensor_copy(g32[:], growf[:])
            grow32[t] = g32
            grow_f[t] = growf

            # interleave compacted rows [-x,-y,-z,rowbits] -> DRAM
            pci = pool.tile([P, KPOS * 4], f32, tag=f"pil{t}", name=f"pci{t}")
            pciv = pci[:].rearrange("p (n c) -> p n c", c=4)
            for c in range(3):
                nc.vector.tensor_scalar_mul(pciv[:, :, c], pcc[t][c], -1.0)
            nc.vector.tensor_copy(pciv[:, :, 3], g32[:].bitcast(f32))
            nc.sync.dma_start(
                out=pc_dram[t][:].rearrange("(p n) c -> p (n c)", p=P),
                in_=pci[:],
            )

        for t in range(TILES):
            stage_a(t)
            vlm_setup(t)
            stage_b(t)
            compose_tau(t)
            compact(t)
        for t in range(TILES):
            compact_finish(t)

        # ---- FPS state ----
        st = []
        for t in range(TILES):
            s = {}
            s["px"], s["py"], s["pz"] = pcc[t]
            s["m8"] = pool.tile([P, 8], f32, tag=f"m8_{t}", name=f"m8_{t}")
            s["pidx"] = pool.tile([P, 8], u32, tag=f"pidx_{t}", name=f"pidx_{t}")
            s["goffi"] = pool.tile([P, 1], i32, tag=f"goffi_{t}", name=f"goffi_{t}")
            s["c4"] = [
                pool.tile([P, 4], f32, tag=f"c4a_{t}", name=f"c4a_{t}"),
                pool.tile([P, 4], f32, tag=f"c4b_{t}", name=f"c4b_{t}"),
            ]
            s["grows"] = pool.tile([P, K], i32, tag=f"grows_{t}", name=f"grows_{t}")
            s["xg"] = pool.tile([P, K * 32], f32, tag=f"xg_{t}", name=f"xg_{t}")
            s["sqx"] = pool.tile([P, KPOS], f32, tag=f"mask{t}", name=f"fsqx{t}")
            s["sqy"] = pool.tile([P, KPOS], f32, tag=f"rank{t}", name=f"fsqy{t}")
            s["sqz"] = pool.tile([P, KPOS], f32, tag=f"am{t}", name=f"fsqz{t}")
            s["s1"] = pool.tile([P, KPOS], f32, tag=f"ch0_{t}", name=f"fs1{t}")
            s["d2"] = pool.tile([P, KPOS], f32, tag=f"ch1_{t}", name=f"fd2{t}")
            s["md"] = [
                pool.tile([P, KPOS], f32, tag=f"ch2_{t}", name=f"mdA_{t}"),
                pool.tile([P, KPOS], f32, tag=f"idx2_{t}", name=f"mdB_{t}"),
            ]
            s["cneg"] = pool.tile([P, 3], f32, tag=f"cneg_{t}", name=f"cneg_{t}")
            st.append(s)

        def squares(t, bias_ap):
            # d2 terms: exact (p - c)^2 via ACT Square(p + (-c)); fp32 FMA
            # inside ACT keeps the subtraction exactly rounded.
            s = st[t]
            nc.scalar.activation(
                s["sqx"][:], s["px"], actf.Square, bias=bias_ap[:, 0:1], scale=1.0
            )
            nc.scalar.activation(
                s["sqy"][:], s["py"], actf.Square, bias=bias_ap[:, 1:2], scale=1.0
            )
            nc.scalar.activation(
                s["sqz"][:], s["pz"], actf.Square, bias=bias_ap[:, 2:3], scale=1.0
            )

        def d2min(t, dst, src):
            # (sqx + sqy) + sqz, then min with current min_d (same order as ref)
            s = st[t]
            nc.gpsimd.tensor_tensor(s["s1"][:], s["sqx"][:], s["sqy"][:], alu.add)
            nc.gpsimd.tensor_tensor(s["d2"][:], s["s1"][:], s["sqz"][:], alu.add)
            nc.vector.tensor_tensor(dst[:], src[:], s["d2"][:], alu.min)

        def record_x(t, k, c4):
            # store pick's global x-row (bits)
            s = st[t]
            nc.vector.tensor_copy(s["grows"][:, k : k + 1], c4[:, 3:4].bitcast(i32))

        def fetch_x(t, k):
            # gather the pick's feature row (emitted late: keeps the Pool
            # queue clear for the chain-critical c4 dispatches)
            s = st[t]
            nc.gpsimd.indirect_dma_start(
                out=s["xg"][:, k * 32 : (k + 1) * 32],
                out_offset=None,
                in_=x_in[:],
                in_offset=IndirectOffsetOnAxis(ap=s["grows"][:, k : k + 1], axis=0),
            )

        # ---- FPS init: start = argmin v over all 1024 points ----
        for t in range(TILES):
            s = st[t]
            b = bst[t]
            vneg = pool.tile([P, NPTS], f32, tag=f"sqy{t}", name=f"vneg{t}")
            nc.vector.tensor_scalar_mul(vneg[:], v[t][:], -1.0)
            nc.vector.max(s["m8"][:], vneg[:])
            nc.vector.max_index(s["pidx"][:], s["m8"][:], vneg[:])
            # global row of the start pick
            nc.vector.tensor_scalar(
                s["grows"][:, 0:1], s["pidx"][:, 0:1].bitcast(i32), xrowf[t][:],
                None, alu.add,
            )
            nc.gpsimd.indirect_dma_start(
                out=s["c4"][0][:, 0:3],
                out_offset=None,
                in_=pos_in[:],
                in_offset=IndirectOffsetOnAxis(ap=s["grows"][:, 0:1], axis=0),
            )
            nc.gpsimd.indirect_dma_start(
                out=s["xg"][:, 0:32],
                out_offset=None,
                in_=x_in[:],
                in_offset=IndirectOffsetOnAxis(ap=s["grows"][:, 0:1], axis=0),
            )
            nc.vector.tensor_scalar_mul(s["cneg"][:], s["c4"][0][:, 0:3], -1.0)
            squares(t, s["cneg"])
        for t in range(TILES):
            s = st[t]
            nc.gpsimd.tensor_tensor(s["s1"][:], s["sqx"][:], s["sqy"][:], alu.add)
            nc.gpsimd.tensor_tensor(s["md"][0][:], s["s1"][:], s["sqz"][:], alu.add)

        # ---- MLP pieces (emitted in chunks between FPS iterations) ----
        xg3 = [st[t]["xg"][:].rearrange("p (k f) -> p k f", f=32) for t in range(TILES)]
        mlp = []
        with tc.tile_pool(name="psum", bufs=1, space="PSUM") as psp:
            # PSUM tiles are shared between the two tiles (bank budget); the
            # Tile dep tracker serializes their MLP chunks, which is fine —
            # the tensor engine is serial anyway.
            ps_xt = psp.tile([P, 1024], f32, tag="psxt")
            ps_h = psp.tile([64, 512], f32, tag="psh")
            ps_h2 = psp.tile([32, 512], f32, tag="psh2")
            ps_z = psp.tile([4, 512], f32, tag="psz")
            ezc = pool.tile([4, 2 * 1024], f32, tag="ezc", name="ezc")
            s4c = pool.tile([4, 2 * 1024], f32, tag="pil1", name="s4c")
            for t in range(TILES):
                m = {}
                m["ps_xt"] = ps_xt
                m["xt4"] = pool.tile([P, 1024], f32, tag=f"xt4_{t}", name=f"xt4_{t}")
                m["ps_h"] = ps_h
                m["h1"] = pool.tile([64, 512], f32, tag=f"h1_{t}", name=f"h1_{t}")
                m["ps_h2"] = ps_h2
                m["h2"] = pool.tile([32, 512], f32, tag=f"h2_{t}", name=f"h2_{t}")
                m["ps_z"] = ps_z
                m["z4"] = pool.tile([4, 1024], f32, tag=f"z4_{t}", name=f"z4_{t}")
                m["ez"] = ezc[:, t * 1024 : (t + 1) * 1024]
                m["s4"] = s4c[:, t * 1024 : (t + 1) * 1024]
                mlp.append(m)

            def mlp_chunk(t, j0, j1):
                # transpose picks j0..j1-1 into ps_xt, then MLP those columns
                m, s = mlp[t], st[t]
                for j in range(j0, j1):
                    lane, grp = j % 4, j // 4
                    nc.tensor.matmul(
                        m["ps_xt"][lane * 32 : (lane + 1) * 32,
                                   grp * 128 : (grp + 1) * 128],
                        xg3[t][:, j, :],
                        eye[:],
                        tile_position=(0, lane * 32),
                    )
                c0, c1 = (j0 // 4) * 128, (j1 // 4) * 128
                w = c1 - c0
                nc.vector.tensor_copy(m["xt4"][:, c0:c1], m["ps_xt"][:, c0:c1])
                nc.tensor.matmul(m["ps_h"][:, 0:w], w1d[:], m["xt4"][:, c0:c1])
                nc.vector.tensor_scalar(
                    m["h1"][:, 0:w], m["ps_h"][:, 0:w], b1d[:], 0.0, alu.add, alu.max
                )
                nc.tensor.matmul(m["ps_h2"][:, 0:w], w2d[:], m["h1"][:, 0:w])
                nc.vector.tensor_scalar(
                    m["h2"][:, 0:w], m["ps_h2"][:, 0:w], b2d[:], 0.0, alu.add, alu.max
                )
                nc.tensor.matmul(m["ps_z"][:, 0:w], w3d[:], m["h2"][:, 0:w])
                nc.vector.tensor_copy(m["z4"][:, c0:c1], m["ps_z"][:, 0:w])
                # softplus(z + b3) = ln(1 + exp(z + b3)). Only the Exp runs
                # per chunk (its table set also serves Square/Identity); all
                # Ln ops are deferred to the tail so the activation table
                # switches exactly once instead of ping-ponging.
                nc.scalar.activation(
                    m["ez"][:, c0:c1], m["z4"][:, c0:c1], actf.Exp,
                    bias=b3d[:], scale=1.0,
                )
                nc.vector.tensor_scalar(
                    m["ez"][:, c0:c1], m["ez"][:, c0:c1], 1.0, None, alu.add
                )
                del c0, c1

            # ---- FPS loop: two tiles software-pipelined, A leads B ----
            A, B = st[0], st[1]
            for t in range(TILES):
                st[t]["zero"] = pool.tile([P, 1], f32, tag=f"zero_{t}", name=f"zero_{t}")

            def argmax_dispatch(s, t, k, md_cur, couple=None):
                nc.vector.max(s["m8"][:], md_cur[:])
                nc.vector.max_index(s["pidx"][:], s["m8"][:], md_cur[:])
                if couple is None:
                    nc.vector.tensor_scalar(
                        s["goffi"][:], s["pidx"][:, 0:1].bitcast(i32), rb512f[:],
                        None, alu.add,
                    )
                else:
                    # phase separation: a zero produced from the partner
                    # tile's in-flight gather delays this dispatch until the
                    # partner's center data has landed, keeping the two
                    # chains' ACT bursts from colliding.
                    nc.vector.tensor_scalar_mul(s["zero"][:], couple[:, 0:1], 0.0)
                    nc.vector.tensor_scalar(
                        s["goffi"][:], s["pidx"][:, 0:1].bitcast(i32), rb512f[:],
                        s["zero"][:], alu.add, alu.add,
                    )
                nc.gpsimd.indirect_dma_start(
                    out=s["c4"][k % 2][:],
                    out_offset=None,
                    in_=pc_dram[t][:],
                    in_offset=IndirectOffsetOnAxis(ap=s["goffi"][:], axis=0),
                )

            def update(s, t, k):
                # c4 rows hold negated coords: biases directly usable.
                # The last pick needs no min_d update at all - only its row.
                c4 = s["c4"][k % 2]
                if k < K - 1:
                    squares(t, c4)
                    d2min(t, s["md"][k % 2], s["md"][(k - 1) % 2])
                record_x(t, k, c4)

            def argmax_last(s, t):
                # final pick: no min_d update and no center gather - only the
                # pick's global x-row, extracted on-chip from growf via a
                # one-hot reduce (saves a DRAM round trip on the tail chain)
                k = K - 1
                nc.vector.max(s["m8"][:], s["md"][(k - 1) % 2][:])
                nc.vector.max_index(s["pidx"][:], s["m8"][:], s["md"][(k - 1) % 2][:])
                pf = s["zero"]
                nc.vector.tensor_copy(pf[:], s["pidx"][:, 0:1])
                oh = s["s1"]
                nc.vector.tensor_scalar(oh[:], io512f[:], pf[:], None, alu.is_equal)
                mg = s["d2"]
                nc.gpsimd.tensor_tensor(mg[:], grow_f[t][:], oh[:], alu.mult)
                gl = s["goffi"]
                glf = s["zero"]
                nc.vector.tensor_reduce(glf[:], mg[:], mybir.AxisListType.X, alu.add)
                nc.vector.tensor_copy(gl[:], glf[:])
                nc.vector.tensor_copy(s["grows"][:, k : k + 1], gl[:])
                fetch_x(t, k)

            for k in range(1, K - 1):
                argmax_dispatch(A, 0, k, A["md"][(k - 1) % 2])
                if k > 1:
                    update(B, 1, k - 1)
                argmax_dispatch(B, 1, k, B["md"][(k - 1) % 2])
                update(A, 0, k)
                if k > 1:
                    fetch_x(1, k - 1)
                fetch_x(0, k)
                if k == 17:
                    mlp_chunk(0, 0, 16)
                elif k == 18:
                    mlp_chunk(1, 0, 16)
                elif k == 25:
                    mlp_chunk(0, 16, 24)
                elif k == 26:
                    mlp_chunk(1, 16, 24)
                elif k == 29:
                    mlp_chunk(0, 24, 28)
                elif k == 30:
                    mlp_chunk(1, 24, 28)
            argmax_last(A, 0)
            update(B, 1, K - 2)
            fetch_x(1, K - 2)
            argmax_last(B, 1)

            # final MLP chunks (picks 28-31 only)
            mlp_chunk(0, 28, 32)
            mlp_chunk(1, 28, 32)

            # ---- tail ----
            # one fused Ln over both tiles' (1 + exp(z + b3)) columns: a
            # single instruction cannot be interleaved with Exp ops by the
            # scheduler, so the activation table switches exactly once.
            nc.scalar.activation(s4c[:], ezc[:], actf.Ln)
            for t in range(TILES):
                s, m = st[t], mlp[t]
                # indices: local = global - row base
                loc = pool.tile([P, K], i32, tag=f"loc_{t}", name=f"loc_{t}")
                nxr = pool.tile([P, 1], f32, tag=f"nxr_{t}", name=f"nxr_{t}")
                nc.gpsimd.tensor_scalar_mul(nxr[:], xrowf[t][:], -1.0)
                nc.vector.tensor_scalar(
                    loc[:], s["grows"][:], nxr[:], None, alu.add
                )
                nc.sync.dma_start(out=i_out[t * P : (t + 1) * P, :], in_=loc[:])

                ps_w = psp.tile([P, K], f32, tag=f"psW{t}")
                for c in range(8):
                    nc.tensor.transpose(
                        ps_w[:, c * 4 : (c + 1) * 4],
                        m["s4"][:, c * 128 : (c + 1) * 128],
                        eye[0:4, 0:4],
                    )
                wout = pool.tile([P, K], f32, tag=f"wout_{t}", name=f"wout_{t}")
                nc.vector.tensor_copy(wout[:], ps_w[:])
                nc.sync.dma_start(out=w_out[t * P : (t + 1) * P, :], in_=wout[:])

    nc.compile()
    return nc


def _host_prep(W1, b1, W2, b2, W3, b3):
    """Block-diagonal 4-lane weight stacks + replicated biases."""
    W1 = np.asarray(W1, np.float32)
    W2 = np.asarray(W2, np.float32)
    W3 = np.asarray(W3, np.float32)
    W1d = np.zeros((128, 64), np.float32)
    W2d = np.zeros((64, 32), np.float32)
    W3d = np.zeros((32, 4), np.float32)
    for l in range(4):
        W1d[l * 32 : (l + 1) * 32, l * 16 : (l + 1) * 16] = W1
        W2d[l * 16 : (l + 1) * 16, l * 8 : (l + 1) * 8] = W2
        W3d[l * 8 : (l + 1) * 8, l : l + 1] = W3
    b1d = np.tile(np.asarray(b1, np.float32), 4).reshape(64, 1)
    b2d = np.tile(np.asarray(b2, np.float32), 4).reshape(32, 1)
    b3d = np.tile(np.asarray(b3, np.float32), 4).reshape(4, 1)
    return W1d, W2d, W3d, b1d, b2d, b3d


_NC = None


def _get_nc():
    global _NC
    if _NC is None:
        _NC = build_nc()
    return _NC


def kernel(x, pos, batch, W1, b1, W2, b2, W3, b3):
    from concourse.bass_utils import run_bass_kernel_spmd

    x = np.ascontiguousarray(np.asarray(x, np.float32))
    pos = np.ascontiguousarray(np.asarray(pos, np.float32))
    W1d, W2d, W3d, b1d, b2d, b3d = _host_prep(W1, b1, W2, b2, W3, b3)
    eye128 = np.eye(128, dtype=np.float32)

    rows = S_CORE * NPTS
    in_maps = []
    for c in range(N_CORES):
        in_maps.append(
            {
                "pos": pos[c * rows : (c + 1) * rows],
                "x": x[c * rows : (c + 1) * rows],
                "W1d": W1d, "W2d": W2d, "W3d": W3d,
                "b1d": b1d, "b2d": b2d, "b3d": b3d,
                "eye128": eye128,
            }
        )

    nc = _get_nc()
    res = run_bass_kernel_spmd(nc, in_maps, list(range(N_CORES))).results
    weights = np.concatenate([res[c]["weights_out"] for c in range(N_CORES)], axis=0)
    indices = np.concatenate(
        [res[c]["indices_out"].astype(np.int32) for c in range(N_CORES)], axis=0
    )
    return weights, indices


# revision 51
# speedup vs baseline: 1.0056x; 1.0056x over previous
"""Trainium2 Bass kernel for nn_Detector (batched FPS detector head).

Pipeline per core (256 submaps = 2 tiles of 128 submaps x 1024 points):
  1. Load pos interleaved (split across DMA queues), split channels.
  2. v = px^2 + py^2 per point (exact f32, matches reference ranking).
  3. Exact 512-smallest threshold per submap via bit-space binary search:
     15 rounds on the u16 high halves of v (4x-packed DVE compares), then
     16 rounds on full f32 v with thresholds composed bitwise from u16
     halves (exact; avoids f32 integer rounding above 2^24).
  4. Stream-compact the 512 selected points per submap (cumsum ranks +
     per-partition local_scatter of coords as u16 halves). Negated coords
     + global x-row index go to DRAM so the FPS loop can fetch its pick's
     center (as ready-made Square biases) via one tiny indirect DMA.
  5. 32-point farthest-point sampling, two tiles software-pipelined:
     d^2 terms are single ACT Square(p + (-c)) ops (exact FMA + square),
     sums/min on Pool/DVE, argmax on DVE. Per pick, the x feature row is
     gathered immediately so the MLP runs in chunks during the loop.
  6. Tiny MLP on the 32 selected rows (block-diagonal weights, 4 lanes),
     relu via exact tensor_scalar max; softplus(z) = ln(1 + exp(z)) with
     per-chunk Exp during the loop and one fused Ln in the tail (exactly
     two activation-table loads in the whole program).

Outputs per core: weights [256, 32] f32, indices [256, 32] int32.
"""

import sys

for _p in ("/opt/trn_rl_repo",):
    if _p not in sys.path:
        sys.path.insert(0, _p)

import numpy as np

import concourse.bass as bass
import concourse.bacc as bacc
import concourse.mybir as mybir
from concourse.bass import IndirectOffsetOnAxis
from concourse.mybir import ActivationFunctionType as actf
from concourse.mybir import AluOpType as alu
from concourse.tile import TileContext

f32 = mybir.dt.float32
i32 = mybir.dt.int32
i16 = mybir.dt.int16
u16 = mybir.dt.uint16
u32 = mybir.dt.uint32

P = 128          # partitions = submaps per tile
NPTS = 1024      # points per submap
KPOS = 512       # closest points kept
K = 32           # FPS samples per submap
TILES = 2        # tiles per core
S_CORE = P * TILES  # submaps per core
N_CORES = 8


def build_nc():
    nc = bacc.Bacc()

    pos_in = nc.declare_dram_parameter("pos", [S_CORE * NPTS, 3], f32, isOutput=False)
    x_in = nc.declare_dram_parameter("x", [S_CORE * NPTS, 32], f32, isOutput=False)
    w1d_in = nc.declare_dram_parameter("W1d", [128, 64], f32, isOutput=False)
    w2d_in = nc.declare_dram_parameter("W2d", [64, 32], f32, isOutput=False)
    w3d_in = nc.declare_dram_parameter("W3d", [32, 4], f32, isOutput=False)
    b1d_in = nc.declare_dram_parameter("b1d", [64, 1], f32, isOutput=False)
    b2d_in = nc.declare_dram_parameter("b2d", [32, 1], f32, isOutput=False)
    b3d_in = nc.declare_dram_parameter("b3d", [4, 1], f32, isOutput=False)
    eye_in = nc.declare_dram_parameter("eye128", [128, 128], f32, isOutput=False)

    w_out = nc.declare_dram_parameter("weights_out", [S_CORE, K], f32, isOutput=True)
    i_out = nc.declare_dram_parameter("indices_out", [S_CORE, K], i32, isOutput=True)

    # DRAM scratch: compacted rows [-x, -y, -z, global_row_bits] per point
    pc_dram = [
        nc.dram_tensor(f"pc_dram{t}", [P * KPOS, 4], f32) for t in range(TILES)
    ]

    pos_t = pos_in[:].rearrange("(t p f) c -> t p (f c)", t=TILES, p=P)

    with TileContext(nc) as tc, tc.tile_pool(name="main", bufs=1) as pool:
        # ---- load pos: three DMA thirds per tile, one per DGE engine ----
        T3 = NPTS
        pil, xrowf, xrowi = [], [], []
        for t in range(TILES):
            p_t = pool.tile([P, NPTS * 3], f32, tag=f"pil{t}", name=f"pil{t}")
            for j, eng in enumerate((nc.sync, nc.scalar, nc.gpsimd)):
                eng.dma_start(
                    out=p_t[:, j * T3 : (j + 1) * T3],
                    in_=pos_t[t][:, j * T3 : (j + 1) * T3],
                )
            pil.append(p_t)
            xr = pool.tile([P, 1], i32, tag=f"xrow{t}", name=f"xrow{t}")
            nc.gpsimd.iota(xr[:], [[1, 1]], base=t * P * NPTS, channel_multiplier=NPTS)
            xrowi.append(xr)
            xrf = pool.tile([P, 1], f32, tag=f"xrowf{t}", name=f"xrowf{t}")
            nc.vector.tensor_copy(xrf[:], xr[:])
            xrowf.append(xrf)

        # ---- shared constants ----
        eye = pool.tile([P, 128], f32, tag="eye")
        nc.sync.dma_start(out=eye[:], in_=eye_in[:])
        rb512 = pool.tile([P, 1], i32, tag="rb512")
        nc.gpsimd.iota(rb512[:], [[1, 1]], base=0, channel_multiplier=KPOS)
        rb512f = pool.tile([P, 1], f32, tag="rb512f")
        nc.vector.tensor_copy(rb512f[:], rb512[:])
        iota16 = pool.tile([P, NPTS], i16, tag="iota16")
        nc.gpsimd.iota(iota16[:], [[1, NPTS]], channel_multiplier=0)
        io512i = pool.tile([P, KPOS], i32, tag="io512i")
        nc.gpsimd.iota(io512i[:], [[1, KPOS]], channel_multiplier=0)
        io512f = pool.tile([P, KPOS], f32, tag="io512f")
        nc.vector.tensor_copy(io512f[:], io512i[:])

        w1d = pool.tile([P, 64], f32, tag="w1d")
        nc.sync.dma_start(out=w1d[:], in_=w1d_in[:])
        w2d = pool.tile([64, 32], f32, tag="w2d")
        nc.sync.dma_start(out=w2d[:], in_=w2d_in[:])
        w3d = pool.tile([32, 4], f32, tag="w3d")
        nc.sync.dma_start(out=w3d[:], in_=w3d_in[:])
        b1d = pool.tile([64, 1], f32, tag="b1d")
        nc.sync.dma_start(out=b1d[:], in_=b1d_in[:])
        b2d = pool.tile([32, 1], f32, tag="b2d")
        nc.sync.dma_start(out=b2d[:], in_=b2d_in[:])
        b3d = pool.tile([4, 1], f32, tag="b3d")
        nc.sync.dma_start(out=b3d[:], in_=b3d_in[:])
        b3n = pool.tile([4, 1], f32, tag="b3n")
        nc.vector.tensor_scalar_mul(b3n[:], b3d[:], -1.0)
        cm1 = pool.tile([P, 1], f32, tag="cm1")
        nc.vector.memset(cm1[:], -1.0)
        cm2 = pool.tile([P, 1], f32, tag="cm2")
        nc.vector.memset(cm2[:], -2.0)

        # ---- split channels + v = px^2 + py^2; u16 high halves of v ----
        v, vh, pch = [], [], []
        for t in range(TILES):
            p3 = pil[t][:].rearrange("p (n c) -> p n c", c=3)
            chans = []
            for c, eng in ((0, nc.gpsimd), (1, nc.vector), (2, nc.scalar)):
                ch = pool.tile([P, NPTS], f32, tag=f"ch{c}_{t}", name=f"ch{c}_{t}")
                if eng is nc.scalar:
                    eng.copy(ch[:], p3[:, :, c])
                else:
                    eng.tensor_copy(ch[:], p3[:, :, c])
                chans.append(ch)
            pch.append(chans)
            sq = pool.tile([P, NPTS], f32, tag=f"sq{t}", name=f"sq{t}")
            nc.gpsimd.tensor_tensor(sq[:], chans[0][:], chans[0][:], alu.mult)
            sqy = pool.tile([P, NPTS], f32, tag=f"sqy{t}", name=f"sqy{t}")
            nc.vector.tensor_tensor(sqy[:], chans[1][:], chans[1][:], alu.mult)
            vt = pool.tile([P, NPTS], f32, tag=f"v{t}", name=f"v{t}")
            nc.gpsimd.tensor_tensor(vt[:], sq[:], sqy[:], alu.add)
            v.append(vt)
            vht = pool.tile([P, NPTS], u16, tag=f"vh{t}", name=f"vh{t}")
            v16 = vt[:].bitcast(u16).rearrange("p (n two) -> p n two", two=2)
            nc.scalar.copy(vht[:], v16[:, :, 1])
            vh.append(vht)

        # ---- bit-space binary search for the 512th-smallest v ----
        # T = bit pattern of the 512th smallest (positive floats are monotone
        # as unsigned ints). Build T bit by bit: lo += 2^i unless
        # count(bits <= lo + 2^i - 1) >= 512. Rounds 30..16 compare only the
        # u16 high halves (exact); rounds 15..0 compare full f32 v against a
        # threshold whose bits are composed from u16 halves (exact compose).
        bst = []
        for t in range(TILES):
            b = {}
            b["loH"] = pool.tile([P, 1], f32, tag=f"loH{t}", name=f"loH{t}")
            nc.vector.memset(b["loH"][:], 0.0)
            b["loL"] = pool.tile([P, 1], f32, tag=f"loL{t}", name=f"loL{t}")
            nc.vector.memset(b["loL"][:], 0.0)
            for nm in ("thrf", "cnt", "cond"):
                b[nm] = pool.tile([P, 1], f32, tag=f"{nm}{t}", name=f"{nm}{t}")
            b["thr"] = pool.tile([P, 1], f32, tag=f"thr{t}", name=f"thr{t}")
            b["junk16"] = pool.tile([P, NPTS], u16, tag=f"junk16_{t}", name=f"junk16_{t}")
            b["junkf"] = pool.tile([P, NPTS], f32, tag=f"sq{t}", name=f"junkf{t}")
            bst.append(b)

        # stage A: 15 rounds on u16 high halves (sign bit of v is 0).
        # Each tile runs its whole search back-to-back (tile 0 first) so
        # tile 0's Pool-heavy compaction overlaps tile 1's DVE-bound search.
        def stage_a(t):
            for i in range(14, -1, -1):
                b = bst[t]
                nc.vector.tensor_scalar(
                    b["thrf"][:], b["loH"][:], float(2**i - 1), None, alu.add
                )
                nc.vector.tensor_scalar(
                    b["junk16"][:], vh[t][:], b["thrf"][:], None,
                    alu.is_le, alu.add, accum_out=b["cnt"][:],
                )
                nc.vector.tensor_scalar(
                    b["cond"][:], b["cnt"][:], 512.0, None, alu.is_lt
                )
                nc.vector.tensor_scalar(
                    b["loH"][:], b["cond"][:], float(2**i), b["loH"][:],
                    alu.mult, alu.add,
                )
        # stage B operates on the u16 LOW halves, masked so that only
        # points whose high half equals H* participate: vlm = vl where
        # vh == loH else 0xFFFF. Probed thresholds never reach 0xFFFF
        # (thr = loL + 2^i - 1 <= 0xFFFF - 2^i), so masked points never
        # count. The target rank becomes 512 - count(vh < loH).
        def vlm_setup(t):
            b = bst[t]
            vlt = pool.tile([P, NPTS], u16, tag=f"junkf{t}x", name=f"vl{t}")
            v16 = v[t][:].bitcast(u16).rearrange("p (n two) -> p n two", two=2)
            nc.scalar.copy(vlt[:], v16[:, :, 0])
            eq = pool.tile([P, NPTS], u16, tag=f"mask{t}", name=f"eq{t}")
            nc.vector.tensor_scalar(eq[:], vh[t][:], b["loH"][:], None, alu.is_equal)
            t1 = pool.tile([P, NPTS], u16, tag=f"rank{t}", name=f"t1_{t}")
            nc.vector.tensor_tensor(t1[:], vlt[:], eq[:], alu.mult)
            t2 = pool.tile([P, NPTS], u16, tag=f"am{t}", name=f"t2_{t}")
            nc.vector.tensor_scalar(t2[:], eq[:], -65535.0, 65535.0, alu.mult, alu.add)
            b["vlm"] = pool.tile([P, NPTS], u16, tag=f"junkf{t}y", name=f"vlm{t}")
            nc.vector.tensor_tensor(b["vlm"][:], t1[:], t2[:], alu.add)
            # target rank among the vh == H* points
            nc.vector.tensor_scalar(
                b["junk16"][:], vh[t][:], b["loH"][:], None,
                alu.is_lt, alu.add, accum_out=b["cnt"][:],
            )
            b["tgt"] = pool.tile([P, 1], f32, tag=f"tgt{t}", name=f"tgt{t}")
            nc.gpsimd.tensor_scalar(
                b["tgt"][:], b["cnt"][:], -1.0, 512.0, alu.mult, alu.add
            )
        def stage_b(t):
            for i in range(15, -1, -1):
                b = bst[t]
                nc.vector.tensor_scalar(
                    b["thrf"][:], b["loL"][:], float(2**i - 1), None, alu.add
                )
                nc.vector.tensor_scalar(
                    b["junk16"][:], b["vlm"][:], b["thrf"][:], None,
                    alu.is_le, alu.add, accum_out=b["cnt"][:],
                )
                nc.vector.tensor_scalar(
                    b["cond"][:], b["cnt"][:], b["tgt"][:], None, alu.is_lt
                )
                nc.vector.tensor_scalar(
                    b["loL"][:], b["cond"][:], float(2**i), b["loL"][:],
                    alu.mult, alu.add,
                )
        def compose_tau(t):
            # tau = bitcast((loH << 16) | loL)
            b = bst[t]
            thr16 = b["thr"][:].bitcast(u16).rearrange("p (n two) -> p n two", two=2)
            nc.vector.tensor_copy(thr16[:, :, 1], b["loH"][:])
            nc.vector.tensor_copy(thr16[:, :, 0], b["loL"][:])

        # ---- compact: mask -> ranks -> scatter coords/indices ----
        pcc, grow32, grow_f = [None, None], [None, None], [None, None]
        sel16s = [None, None]

        def compact(t):
            b = bst[t]
            mask = pool.tile([P, NPTS], f32, tag=f"mask{t}", name=f"mask{t}")
            nc.vector.tensor_scalar(mask[:], v[t][:], b["thr"][:], None, alu.is_le)
            rank = pool.tile([P, NPTS], f32, tag=f"rank{t}", name=f"rank{t}")
            nc.vector.tensor_tensor_scan(
                rank[:], mask[:], mask[:], 0.0, alu.add, alu.bypass
            )
            am = pool.tile([P, NPTS], f32, tag=f"am{t}", name=f"am{t}")
            nc.vector.tensor_tensor(am[:], rank[:], mask[:], alu.mult)

            # u16-half slot indices: even half -> 2a-2, odd half -> 2a-1
            idx2 = pool.tile([P, 2 * NPTS], i16, tag=f"idx2_{t}", name=f"idx2_{t}")
            i2v = idx2[:].rearrange("p (n two) -> p n two", two=2)
            nc.scalar.activation(i2v[:, :, 0], am[:], actf.Identity, bias=cm2[:], scale=2.0)
            nc.scalar.activation(i2v[:, :, 1], am[:], actf.Identity, bias=cm1[:], scale=2.0)
            slot16 = pool.tile([P, NPTS], i16, tag=f"slot16{t}", name=f"slot16{t}")
            nc.scalar.activation(slot16[:], am[:], actf.Identity, bias=cm1[:], scale=1.0)

            # compacted coord channels (as u16 halves of f32)
            chc = []
            for c in range(3):
                cc = pool.tile([P, 2 * KPOS], u16, tag=f"cc{c}_{t}", name=f"cc{c}_{t}")
                nc.gpsimd.local_scatter(
                    cc[:], pch[t][c][:].bitcast(u16), idx2[:],
                    channels=P, num_elems=2 * KPOS, num_idxs=2 * NPTS,
                )
                chc.append(cc)
            pcc[t] = [cc[:].bitcast(f32) for cc in chc]

            # compacted original indices (i16 scatter of iota, then widen)
            sel16 = pool.tile([P, KPOS], i16, tag=f"sel16{t}", name=f"sel16{t}")
            nc.gpsimd.local_scatter(
                sel16[:], iota16[:], slot16[:],
                channels=P, num_elems=KPOS, num_idxs=NPTS,
            )
            sel16s[t] = sel16

        def compact_finish(t):
            # row-index conversion + DRAM scratch write: gates only the c4
            # gathers of the FPS loop, so it runs after both tiles' searches
            sel16 = sel16s[t]
            self_f = pool.tile([P, KPOS], f32, tag=f"selff{t}", name=f"selff{t}")
            nc.vector.tensor_copy(self_f[:], sel16[:])
            growf = pool.tile([P, KPOS], f32, tag=f"growf{t}", name=f"growf{t}")
            nc.vector.tensor_scalar(growf[:], self_f[:], xrowf[t][:], None, alu.add)
            g32 = pool.tile([P, KPOS], i32, tag=f"g32_{t}", name=f"g32_{t}")
            nc.vector.tensor_copy(g32[:], growf[:])
            grow32[t] = g32
            grow_f[t] = growf

            # interleave compacted rows [-x,-y,-z,rowbits] -> DRAM
            pci = pool.tile([P, KPOS * 4], f32, tag=f"pil{t}", name=f"pci{t}")
            pciv = pci[:].rearrange("p (n c) -> p n c", c=4)
            for c in range(3):
                nc.vector.tensor_scalar_mul(pciv[:, :, c], pcc[t][c], -1.0)
            nc.vector.tensor_copy(pciv[:, :, 3], g32[:].bitcast(f32))
            nc.sync.dma_start(
                out=pc_dram[t][:].rearrange("(p n) c -> p (n c)", p=P),
                in_=pci[:],
            )

        for t in range(TILES):
            stage_a(t)
            vlm_setup(t)
            stage_b(t)
            compose_tau(t)
            compact(t)
        for t in range(TILES):
            compact_finish(t)

        # ---- FPS state ----
        st = []
        for t in range(TILES):
            s = {}
            s["px"], s["py"], s["pz"] = pcc[t]
            s["m8"] = pool.tile([P, 8], f32, tag=f"m8_{t}", name=f"m8_{t}")
            s["pidx"] = pool.tile([P, 8], u32, tag=f"pidx_{t}", name=f"pidx_{t}")
            s["goffi"] = pool.tile([P, 1], i32, tag=f"goffi_{t}", name=f"goffi_{t}")
            s["c4"] = [
                pool.tile([P, 4], f32, tag=f"c4a_{t}", name=f"c4a_{t}"),
                pool.tile([P, 4], f32, tag=f"c4b_{t}", name=f"c4b_{t}"),
            ]
            s["grows"] = pool.tile([P, K], i32, tag=f"grows_{t}", name=f"grows_{t}")
            s["xg"] = pool.tile([P, K * 32], f32, tag=f"xg_{t}", name=f"xg_{t}")
            s["sqx"] = pool.tile([P, KPOS], f32, tag=f"mask{t}", name=f"fsqx{t}")
            s["sqy"] = pool.tile([P, KPOS], f32, tag=f"rank{t}", name=f"fsqy{t}")
            s["sqz"] = pool.tile([P, KPOS], f32, tag=f"am{t}", name=f"fsqz{t}")
            s["s1"] = pool.tile([P, KPOS], f32, tag=f"ch0_{t}", name=f"fs1{t}")
            s["d2"] = pool.tile([P, KPOS], f32, tag=f"ch1_{t}", name=f"fd2{t}")
            s["md"] = [
                pool.tile([P, KPOS], f32, tag=f"ch2_{t}", name=f"mdA_{t}"),
                pool.tile([P, KPOS], f32, tag=f"idx2_{t}", name=f"mdB_{t}"),
            ]
            s["cneg"] = pool.tile([P, 3], f32, tag=f"cneg_{t}", name=f"cneg_{t}")
            st.append(s)

        def squares(t, bias_ap):
            # d2 terms: exact (p - c)^2 via ACT Square(p + (-c)); fp32 FMA
            # inside ACT keeps the subtraction exactly rounded.
            s = st[t]
            nc.scalar.activation(
                s["sqx"][:], s["px"], actf.Square, bias=bias_ap[:, 0:1], scale=1.0
            )
            nc.scalar.activation(
                s["sqy"][:], s["py"], actf.Square, bias=bias_ap[:, 1:2], scale=1.0
            )
            nc.scalar.activation(
                s["sqz"][:], s["pz"], actf.Square, bias=bias_ap[:, 2:3], scale=1.0
            )

        def d2min(t, dst, src):
            # (sqx + sqy) + sqz, then min with current min_d (same order as ref)
            s = st[t]
            nc.gpsimd.tensor_tensor(s["s1"][:], s["sqx"][:], s["sqy"][:], alu.add)
            nc.gpsimd.tensor_tensor(s["d2"][:], s["s1"][:], s["sqz"][:], alu.add)
            nc.vector.tensor_tensor(dst[:], src[:], s["d2"][:], alu.min)

        def record_x(t, k, c4):
            # store pick's global x-row (bits)
            s = st[t]
            nc.vector.tensor_copy(s["grows"][:, k : k + 1], c4[:, 3:4].bitcast(i32))

        def fetch_x(t, k):
            # gather the pick's feature row (emitted late: keeps the Pool
            # queue clear for the chain-critical c4 dispatches)
            s = st[t]
            nc.gpsimd.indirect_dma_start(
                out=s["xg"][:, k * 32 : (k + 1) * 32],
                out_offset=None,
                in_=x_in[:],
                in_offset=IndirectOffsetOnAxis(ap=s["grows"][:, k : k + 1], axis=0),
            )

        # ---- FPS init: start = argmin v over all 1024 points ----
        for t in range(TILES):
            s = st[t]
            b = bst[t]
            vneg = pool.tile([P, NPTS], f32, tag=f"sqy{t}", name=f"vneg{t}")
            nc.vector.tensor_scalar_mul(vneg[:], v[t][:], -1.0)
            nc.vector.max(s["m8"][:], vneg[:])
            nc.vector.max_index(s["pidx"][:], s["m8"][:], vneg[:])
            # global row of the start pick
            nc.vector.tensor_scalar(
                s["grows"][:, 0:1], s["pidx"][:, 0:1].bitcast(i32), xrowf[t][:],
                None, alu.add,
            )
            nc.gpsimd.indirect_dma_start(
                out=s["c4"][0][:, 0:3],
                out_offset=None,
                in_=pos_in[:],
                in_offset=IndirectOffsetOnAxis(ap=s["grows"][:, 0:1], axis=0),
            )
            nc.gpsimd.indirect_dma_start(
                out=s["xg"][:, 0:32],
                out_offset=None,
                in_=x_in[:],
                in_offset=IndirectOffsetOnAxis(ap=s["grows"][:, 0:1], axis=0),
            )
            nc.vector.tensor_scalar_mul(s["cneg"][:], s["c4"][0][:, 0:3], -1.0)
            squares(t, s["cneg"])
        for t in range(TILES):
            s = st[t]
            nc.gpsimd.tensor_tensor(s["s1"][:], s["sqx"][:], s["sqy"][:], alu.add)
            nc.gpsimd.tensor_tensor(s["md"][0][:], s["s1"][:], s["sqz"][:], alu.add)

        # ---- MLP pieces (emitted in chunks between FPS iterations) ----
        xg3 = [st[t]["xg"][:].rearrange("p (k f) -> p k f", f=32) for t in range(TILES)]
        mlp = []
        with tc.tile_pool(name="psum", bufs=1, space="PSUM") as psp:
            # PSUM tiles are shared between the two tiles (bank budget); the
            # Tile dep tracker serializes their MLP chunks, which is fine —
            # the tensor engine is serial anyway.
            ps_xt = psp.tile([P, 1024], f32, tag="psxt")
            ps_h = psp.tile([64, 512], f32, tag="psh")
            ps_h2 = psp.tile([32, 512], f32, tag="psh2")
            ps_z = psp.tile([4, 512], f32, tag="psz")
            ezc = pool.tile([4, 2 * 1024], f32, tag="ezc", name="ezc")
            s4c = pool.tile([4, 2 * 1024], f32, tag="pil1", name="s4c")
            for t in range(TILES):
                m = {}
                m["ps_xt"] = ps_xt
                m["xt4"] = pool.tile([P, 1024], f32, tag=f"xt4_{t}", name=f"xt4_{t}")
                m["ps_h"] = ps_h
                m["h1"] = pool.tile([64, 512], f32, tag=f"h1_{t}", name=f"h1_{t}")
                m["ps_h2"] = ps_h2
                m["h2"] = pool.tile([32, 512], f32, tag=f"h2_{t}", name=f"h2_{t}")
                m["ps_z"] = ps_z
                m["z4"] = pool.tile([4, 1024], f32, tag=f"z4_{t}", name=f"z4_{t}")
                m["ez"] = ezc[:, t * 1024 : (t + 1) * 1024]
                m["s4"] = s4c[:, t * 1024 : (t + 1) * 1024]
                mlp.append(m)

            def mlp_chunk(t, j0, j1):
                # transpose picks j0..j1-1 into ps_xt, then MLP those columns
                m, s = mlp[t], st[t]
                for j in range(j0, j1):
                    lane, grp = j % 4, j // 4
                    nc.tensor.matmul(
                        m["ps_xt"][lane * 32 : (lane + 1) * 32,
                                   grp * 128 : (grp + 1) * 128],
                        xg3[t][:, j, :],
                        eye[:],
                        tile_position=(0, lane * 32),
                    )
                c0, c1 = (j0 // 4) * 128, (j1 // 4) * 128
                w = c1 - c0
                nc.vector.tensor_copy(m["xt4"][:, c0:c1], m["ps_xt"][:, c0:c1])
                nc.tensor.matmul(m["ps_h"][:, 0:w], w1d[:], m["xt4"][:, c0:c1])
                nc.vector.tensor_scalar(
                    m["h1"][:, 0:w], m["ps_h"][:, 0:w], b1d[:], 0.0, alu.add, alu.max
                )
                nc.tensor.matmul(m["ps_h2"][:, 0:w], w2d[:], m["h1"][:, 0:w])
                nc.vector.tensor_scalar(
                    m["h2"][:, 0:w], m["ps_h2"][:, 0:w], b2d[:], 0.0, alu.add, alu.max
                )
                nc.tensor.matmul(m["ps_z"][:, 0:w], w3d[:], m["h2"][:, 0:w])
                nc.vector.tensor_copy(m["z4"][:, c0:c1], m["ps_z"][:, 0:w])
                # softplus(z + b3) = ln(1 + exp(z + b3)). Only the Exp runs
                # per chunk (its table set also serves Square/Identity); all
                # Ln ops are deferred to the tail so the activation table
                # switches exactly once instead of ping-ponging.
                nc.scalar.activation(
                    m["ez"][:, c0:c1], m["z4"][:, c0:c1], actf.Exp,
                    bias=b3d[:], scale=1.0,
                )
                nc.vector.tensor_scalar(
                    m["ez"][:, c0:c1], m["ez"][:, c0:c1], 1.0, None, alu.add
                )
                del c0, c1

            # ---- FPS loop: two tiles software-pipelined, A leads B ----
            A, B = st[0], st[1]
            for t in range(TILES):
                st[t]["zero"] = pool.tile([P, 1], f32, tag=f"zero_{t}", name=f"zero_{t}")

            def argmax_dispatch(s, t, k, md_cur, couple=None):
                nc.vector.max(s["m8"][:], md_cur[:])
                nc.vector.max_index(s["pidx"][:], s["m8"][:], md_cur[:])
                if couple is None:
                    nc.vector.tensor_scalar(
                        s["goffi"][:], s["pidx"][:, 0:1].bitcast(i32), rb512f[:],
                        None, alu.add,
                    )
                else:
                    # phase separation: a zero produced from the partner
                    # tile's in-flight gather delays this dispatch until the
                    # partner's center data has landed, keeping the two
                    # chains' ACT bursts from colliding.
                    nc.vector.tensor_scalar_mul(s["zero"][:], couple[:, 0:1], 0.0)
                    nc.vector.tensor_scalar(
                        s["goffi"][:], s["pidx"][:, 0:1].bitcast(i32), rb512f[:],
                        s["zero"][:], alu.add, alu.add,
                    )
                nc.gpsimd.indirect_dma_start(
                    out=s["c4"][k % 2][:],
                    out_offset=None,
                    in_=pc_dram[t][:],
                    in_offset=IndirectOffsetOnAxis(ap=s["goffi"][:], axis=0),
                )

            def update(s, t, k):
                # c4 rows hold negated coords: biases directly usable.
                # The last pick needs no min_d update at all - only its row.
                c4 = s["c4"][k % 2]
                if k < K - 1:
                    squares(t, c4)
                    d2min(t, s["md"][k % 2], s["md"][(k - 1) % 2])
                record_x(t, k, c4)

            def argmax_last(s, t):
                # final pick: no min_d update and no center gather - only the
                # pick's global x-row, extracted on-chip from growf via a
                # one-hot reduce (saves a DRAM round trip on the tail chain)
                k = K - 1
                nc.vector.max(s["m8"][:], s["md"][(k - 1) % 2][:])
                nc.vector.max_index(s["pidx"][:], s["m8"][:], s["md"][(k - 1) % 2][:])
                pf = s["zero"]
                nc.vector.tensor_copy(pf[:], s["pidx"][:, 0:1])
                oh = s["s1"]
                nc.vector.tensor_scalar(oh[:], io512f[:], pf[:], None, alu.is_equal)
                mg = s["d2"]
                nc.gpsimd.tensor_tensor(mg[:], grow_f[t][:], oh[:], alu.mult)
                gl = s["goffi"]
                glf = s["zero"]
                nc.vector.tensor_reduce(glf[:], mg[:], mybir.AxisListType.X, alu.add)
                nc.vector.tensor_copy(gl[:], glf[:])
                nc.vector.tensor_copy(s["grows"][:, k : k + 1], gl[:])
                fetch_x(t, k)

            for k in range(1, K - 1):
                argmax_dispatch(A, 0, k, A["md"][(k - 1) % 2])
                if k > 1:
                    update(B, 1, k - 1)
                argmax_dispatch(B, 1, k, B["md"][(k - 1) % 2])
                update(A, 0, k)
                if k > 1:
                    fetch_x(1, k - 1)
                fetch_x(0, k)
                if k == 17:
                    mlp_chunk(0, 0, 16)
                elif k == 18:
                    mlp_chunk(1, 0, 16)
                elif k == 25:
                    mlp_chunk(0, 16, 24)
                elif k == 26:
                    mlp_chunk(1, 16, 24)
                elif k == 29:
                    mlp_chunk(0, 24, 28)
                elif k == 30:
                    mlp_chunk(1, 24, 28)
            argmax_last(A, 0)
            update(B, 1, K - 2)
            fetch_x(1, K - 2)
            argmax_last(B, 1)

            # final MLP chunks (picks 28-31 only)
            mlp_chunk(0, 28, 32)
            mlp_chunk(1, 28, 32)

            # ---- tail ----
            # one fused Ln over both tiles' (1 + exp(z + b3)) columns: a
            # single instruction cannot be interleaved with Exp ops by the
            # scheduler, so the activation table switches exactly once.
            nc.scalar.activation(s4c[:], ezc[:], actf.Ln)
            for t in range(TILES):
                s, m = st[t], mlp[t]
                # indices: local = global - row base
                loc = pool.tile([P, K], i32, tag=f"loc_{t}", name=f"loc_{t}")
                nxr = pool.tile([P, 1], f32, tag=f"nxr_{t}", name=f"nxr_{t}")
                nc.gpsimd.tensor_scalar_mul(nxr[:], xrowf[t][:], -1.0)
                nc.vector.tensor_scalar(
                    loc[:], s["grows"][:], nxr[:], None, alu.add
                )
                nc.sync.dma_start(out=i_out[t * P : (t + 1) * P, :], in_=loc[:])

                ps_w = psp.tile([P, K], f32, tag=f"psW{t}")
                for c in range(8):
                    nc.tensor.transpose(
                        ps_w[:, c * 4 : (c + 1) * 4],
                        m["s4"][:, c * 128 : (c + 1) * 128],
                        eye[0:4, 0:4],
                    )
                wout = pool.tile([P, K], f32, tag=f"wout_{t}", name=f"wout_{t}")
                nc.vector.tensor_copy(wout[:], ps_w[:])
                nc.sync.dma_start(out=w_out[t * P : (t + 1) * P, :], in_=wout[:])

    nc.compile()
    return nc


def _host_prep(W1, b1, W2, b2, W3, b3):
    """Block-diagonal 4-lane weight stacks + replicated biases."""
    W1 = np.asarray(W1, np.float32)
    W2 = np.asarray(W2, np.float32)
    W3 = np.asarray(W3, np.float32)
    W1d = np.zeros((128, 64), np.float32)
    W2d = np.zeros((64, 32), np.float32)
    W3d = np.zeros((32, 4), np.float32)
    for l in range(4):
        W1d[l * 32 : (l + 1) * 32, l * 16 : (l + 1) * 16] = W1
        W2d[l * 16 : (l + 1) * 16, l * 8 : (l + 1) * 8] = W2
        W3d[l * 8 : (l + 1) * 8, l : l + 1] = W3
    b1d = np.tile(np.asarray(b1, np.float32), 4).reshape(64, 1)
    b2d = np.tile(np.asarray(b2, np.float32), 4).reshape(32, 1)
    b3d = np.tile(np.asarray(b3, np.float32), 4).reshape(4, 1)
    return W1d, W2d, W3d, b1d, b2d, b3d


_NC = None


def _get_nc():
    global _NC
    if _NC is None:
        _NC = build_nc()
    return _NC


def kernel(x, pos, batch, W1, b1, W2, b2, W3, b3):
    from concourse.bass_utils import run_bass_kernel_spmd

    x = np.ascontiguousarray(np.asarray(x, np.float32))
    pos = np.ascontiguousarray(np.asarray(pos, np.float32))
    W1d, W2d, W3d, b1d, b2d, b3d = _host_prep(W1, b1, W2, b2, W3, b3)
    eye128 = np.eye(128, dtype=np.float32)

    rows = S_CORE * NPTS
    in_maps = []
    for c in range(N_CORES):
        in_maps.append(
            {
                "pos": pos[c * rows : (c + 1) * rows],
                "x": x[c * rows : (c + 1) * rows],
                "W1d": W1d, "W2d": W2d, "W3d": W3d,
                "b1d": b1d, "b2d": b2d, "b3d": b3d,
                "eye128": eye128,
            }
        )

    nc = _get_nc()
    res = run_bass_kernel_spmd(nc, in_maps, list(range(N_CORES))).results
    weights = np.concatenate([res[c]["weights_out"] for c in range(N_CORES)], axis=0)
    indices = np.concatenate(
        [res[c]["indices_out"].astype(np.int32) for c in range(N_CORES)], axis=0
    )
    return weights, indices


# revision 52
# speedup vs baseline: 1.0085x; 1.0029x over previous
"""Trainium2 Bass kernel for nn_Detector (batched FPS detector head).

Pipeline per core (256 submaps = 2 tiles of 128 submaps x 1024 points):
  1. Load pos interleaved (split across DMA queues), split channels.
  2. v = px^2 + py^2 per point (exact f32, matches reference ranking).
  3. Exact 512-smallest threshold per submap via bit-space binary search:
     15 rounds on the u16 high halves of v (4x-packed DVE compares), then
     16 rounds on full f32 v with thresholds composed bitwise from u16
     halves (exact; avoids f32 integer rounding above 2^24).
  4. Stream-compact the 512 selected points per submap (cumsum ranks +
     per-partition local_scatter of coords as u16 halves). Negated coords
     + global x-row index go to DRAM so the FPS loop can fetch its pick's
     center (as ready-made Square biases) via one tiny indirect DMA.
  5. 32-point farthest-point sampling, two tiles software-pipelined:
     d^2 terms are single ACT Square(p + (-c)) ops (exact FMA + square),
     sums/min on Pool/DVE, argmax on DVE. Per pick, the x feature row is
     gathered immediately so the MLP runs in chunks during the loop.
  6. Tiny MLP on the 32 selected rows (block-diagonal weights, 4 lanes),
     relu via exact tensor_scalar max; softplus(z) = ln(1 + exp(z)) with
     per-chunk Exp during the loop and one fused Ln in the tail (exactly
     two activation-table loads in the whole program).

Outputs per core: weights [256, 32] f32, indices [256, 32] int32.
"""

import sys

for _p in ("/opt/trn_rl_repo",):
    if _p not in sys.path:
        sys.path.insert(0, _p)

import numpy as np

import concourse.bass as bass
import concourse.bacc as bacc
import concourse.mybir as mybir
from concourse.bass import IndirectOffsetOnAxis
from concourse.mybir import ActivationFunctionType as actf
from concourse.mybir import AluOpType as alu
from concourse.tile import TileContext

f32 = mybir.dt.float32
i32 = mybir.dt.int32
i16 = mybir.dt.int16
u16 = mybir.dt.uint16
u32 = mybir.dt.uint32

P = 128          # partitions = submaps per tile
NPTS = 1024      # points per submap
KPOS = 512       # closest points kept
K = 32           # FPS samples per submap
TILES = 2        # tiles per core
S_CORE = P * TILES  # submaps per core
N_CORES = 8


def build_nc():
    nc = bacc.Bacc()

    pos_in = nc.declare_dram_parameter("pos", [S_CORE * NPTS, 3], f32, isOutput=False)
    x_in = nc.declare_dram_parameter("x", [S_CORE * NPTS, 32], f32, isOutput=False)
    w1d_in = nc.declare_dram_parameter("W1d", [128, 64], f32, isOutput=False)
    w2d_in = nc.declare_dram_parameter("W2d", [64, 32], f32, isOutput=False)
    w3d_in = nc.declare_dram_parameter("W3d", [32, 4], f32, isOutput=False)
    b1d_in = nc.declare_dram_parameter("b1d", [64, 1], f32, isOutput=False)
    b2d_in = nc.declare_dram_parameter("b2d", [32, 1], f32, isOutput=False)
    b3d_in = nc.declare_dram_parameter("b3d", [4, 1], f32, isOutput=False)
    eye_in = nc.declare_dram_parameter("eye128", [128, 128], f32, isOutput=False)

    w_out = nc.declare_dram_parameter("weights_out", [S_CORE, K], f32, isOutput=True)
    i_out = nc.declare_dram_parameter("indices_out", [S_CORE, K], i32, isOutput=True)

    # DRAM scratch: compacted rows [-x, -y, -z, global_row_bits] per point
    pc_dram = [
        nc.dram_tensor(f"pc_dram{t}", [P * KPOS, 4], f32) for t in range(TILES)
    ]

    pos_t = pos_in[:].rearrange("(t p f) c -> t p (f c)", t=TILES, p=P)

    with TileContext(nc) as tc, tc.tile_pool(name="main", bufs=1) as pool:
        # ---- load pos: three DMA thirds per tile, one per DGE engine ----
        T3 = NPTS
        pil, xrowf, xrowi = [], [], []
        for t in range(TILES):
            p_t = pool.tile([P, NPTS * 3], f32, tag=f"pil{t}", name=f"pil{t}")
            for j, eng in enumerate((nc.sync, nc.scalar, nc.gpsimd)):
                eng.dma_start(
                    out=p_t[:, j * T3 : (j + 1) * T3],
                    in_=pos_t[t][:, j * T3 : (j + 1) * T3],
                )
            pil.append(p_t)
            xr = pool.tile([P, 1], i32, tag=f"xrow{t}", name=f"xrow{t}")
            nc.gpsimd.iota(xr[:], [[1, 1]], base=t * P * NPTS, channel_multiplier=NPTS)
            xrowi.append(xr)
            xrf = pool.tile([P, 1], f32, tag=f"xrowf{t}", name=f"xrowf{t}")
            nc.vector.tensor_copy(xrf[:], xr[:])
            xrowf.append(xrf)

        # ---- split channels + v = px^2 + py^2; u16 high halves of v ----
        v, vh, pch = [], [], []
        for t in range(TILES):
            p3 = pil[t][:].rearrange("p (n c) -> p n c", c=3)
            chans = []
            for c, eng in ((0, nc.gpsimd), (1, nc.vector), (2, nc.scalar)):
                ch = pool.tile([P, NPTS], f32, tag=f"ch{c}_{t}", name=f"ch{c}_{t}")
                if eng is nc.scalar:
                    eng.copy(ch[:], p3[:, :, c])
                else:
                    eng.tensor_copy(ch[:], p3[:, :, c])
                chans.append(ch)
            pch.append(chans)
            sq = pool.tile([P, NPTS], f32, tag=f"sq{t}", name=f"sq{t}")
            nc.gpsimd.tensor_tensor(sq[:], chans[0][:], chans[0][:], alu.mult)
            sqy = pool.tile([P, NPTS], f32, tag=f"sqy{t}", name=f"sqy{t}")
            nc.vector.tensor_tensor(sqy[:], chans[1][:], chans[1][:], alu.mult)
            vt = pool.tile([P, NPTS], f32, tag=f"v{t}", name=f"v{t}")
            nc.gpsimd.tensor_tensor(vt[:], sq[:], sqy[:], alu.add)
            v.append(vt)
            vht = pool.tile([P, NPTS], u16, tag=f"vh{t}", name=f"vh{t}")
            v16 = vt[:].bitcast(u16).rearrange("p (n two) -> p n two", two=2)
            nc.scalar.copy(vht[:], v16[:, :, 1])
            vh.append(vht)

        # ---- shared constants ----
        eye = pool.tile([P, 128], f32, tag="eye")
        nc.sync.dma_start(out=eye[:], in_=eye_in[:])
        rb512 = pool.tile([P, 1], i32, tag="rb512")
        nc.gpsimd.iota(rb512[:], [[1, 1]], base=0, channel_multiplier=KPOS)
        rb512f = pool.tile([P, 1], f32, tag="rb512f")
        nc.vector.tensor_copy(rb512f[:], rb512[:])
        iota16 = pool.tile([P, NPTS], i16, tag="iota16")
        nc.gpsimd.iota(iota16[:], [[1, NPTS]], channel_multiplier=0)
        io512i = pool.tile([P, KPOS], i32, tag="io512i")
        nc.gpsimd.iota(io512i[:], [[1, KPOS]], channel_multiplier=0)
        io512f = pool.tile([P, KPOS], f32, tag="io512f")
        nc.vector.tensor_copy(io512f[:], io512i[:])

        w1d = pool.tile([P, 64], f32, tag="w1d")
        nc.sync.dma_start(out=w1d[:], in_=w1d_in[:])
        w2d = pool.tile([64, 32], f32, tag="w2d")
        nc.sync.dma_start(out=w2d[:], in_=w2d_in[:])
        w3d = pool.tile([32, 4], f32, tag="w3d")
        nc.sync.dma_start(out=w3d[:], in_=w3d_in[:])
        b1d = pool.tile([64, 1], f32, tag="b1d")
        nc.sync.dma_start(out=b1d[:], in_=b1d_in[:])
        b2d = pool.tile([32, 1], f32, tag="b2d")
        nc.sync.dma_start(out=b2d[:], in_=b2d_in[:])
        b3d = pool.tile([4, 1], f32, tag="b3d")
        nc.sync.dma_start(out=b3d[:], in_=b3d_in[:])
        b3n = pool.tile([4, 1], f32, tag="b3n")
        nc.vector.tensor_scalar_mul(b3n[:], b3d[:], -1.0)
        cm1 = pool.tile([P, 1], f32, tag="cm1")
        nc.vector.memset(cm1[:], -1.0)
        cm2 = pool.tile([P, 1], f32, tag="cm2")
        nc.vector.memset(cm2[:], -2.0)

        # ---- bit-space binary search for the 512th-smallest v ----
        # T = bit pattern of the 512th smallest (positive floats are monotone
        # as unsigned ints). Build T bit by bit: lo += 2^i unless
        # count(bits <= lo + 2^i - 1) >= 512. Rounds 30..16 compare only the
        # u16 high halves (exact); rounds 15..0 compare full f32 v against a
        # threshold whose bits are composed from u16 halves (exact compose).
        bst = []
        for t in range(TILES):
            b = {}
            b["loH"] = pool.tile([P, 1], f32, tag=f"loH{t}", name=f"loH{t}")
            nc.vector.memset(b["loH"][:], 0.0)
            b["loL"] = pool.tile([P, 1], f32, tag=f"loL{t}", name=f"loL{t}")
            nc.vector.memset(b["loL"][:], 0.0)
            for nm in ("thrf", "cnt", "cond"):
                b[nm] = pool.tile([P, 1], f32, tag=f"{nm}{t}", name=f"{nm}{t}")
            b["thr"] = pool.tile([P, 1], f32, tag=f"thr{t}", name=f"thr{t}")
            b["junk16"] = pool.tile([P, NPTS], u16, tag=f"junk16_{t}", name=f"junk16_{t}")
            b["junkf"] = pool.tile([P, NPTS], f32, tag=f"sq{t}", name=f"junkf{t}")
            bst.append(b)

        # stage A: 15 rounds on u16 high halves (sign bit of v is 0).
        # Each tile runs its whole search back-to-back (tile 0 first) so
        # tile 0's Pool-heavy compaction overlaps tile 1's DVE-bound search.
        def stage_a(t):
            for i in range(14, -1, -1):
                b = bst[t]
                nc.vector.tensor_scalar(
                    b["thrf"][:], b["loH"][:], float(2**i - 1), None, alu.add
                )
                nc.vector.tensor_scalar(
                    b["junk16"][:], vh[t][:], b["thrf"][:], None,
                    alu.is_le, alu.add, accum_out=b["cnt"][:],
                )
                nc.vector.tensor_scalar(
                    b["cond"][:], b["cnt"][:], 512.0, None, alu.is_lt
                )
                nc.vector.tensor_scalar(
                    b["loH"][:], b["cond"][:], float(2**i), b["loH"][:],
                    alu.mult, alu.add,
                )
        # stage B operates on the u16 LOW halves, masked so that only
        # points whose high half equals H* participate: vlm = vl where
        # vh == loH else 0xFFFF. Probed thresholds never reach 0xFFFF
        # (thr = loL + 2^i - 1 <= 0xFFFF - 2^i), so masked points never
        # count. The target rank becomes 512 - count(vh < loH).
        def vlm_setup(t):
            b = bst[t]
            vlt = pool.tile([P, NPTS], u16, tag=f"junkf{t}x", name=f"vl{t}")
            v16 = v[t][:].bitcast(u16).rearrange("p (n two) -> p n two", two=2)
            nc.scalar.copy(vlt[:], v16[:, :, 0])
            eq = pool.tile([P, NPTS], u16, tag=f"mask{t}", name=f"eq{t}")
            nc.vector.tensor_scalar(eq[:], vh[t][:], b["loH"][:], None, alu.is_equal)
            t1 = pool.tile([P, NPTS], u16, tag=f"rank{t}", name=f"t1_{t}")
            nc.vector.tensor_tensor(t1[:], vlt[:], eq[:], alu.mult)
            t2 = pool.tile([P, NPTS], u16, tag=f"am{t}", name=f"t2_{t}")
            nc.vector.tensor_scalar(t2[:], eq[:], -65535.0, 65535.0, alu.mult, alu.add)
            b["vlm"] = pool.tile([P, NPTS], u16, tag=f"junkf{t}y", name=f"vlm{t}")
            nc.vector.tensor_tensor(b["vlm"][:], t1[:], t2[:], alu.add)
            # target rank among the vh == H* points
            nc.vector.tensor_scalar(
                b["junk16"][:], vh[t][:], b["loH"][:], None,
                alu.is_lt, alu.add, accum_out=b["cnt"][:],
            )
            b["tgt"] = pool.tile([P, 1], f32, tag=f"tgt{t}", name=f"tgt{t}")
            nc.gpsimd.tensor_scalar(
                b["tgt"][:], b["cnt"][:], -1.0, 512.0, alu.mult, alu.add
            )
        def stage_b(t):
            for i in range(15, -1, -1):
                b = bst[t]
                nc.vector.tensor_scalar(
                    b["thrf"][:], b["loL"][:], float(2**i - 1), None, alu.add
                )
                nc.vector.tensor_scalar(
                    b["junk16"][:], b["vlm"][:], b["thrf"][:], None,
                    alu.is_le, alu.add, accum_out=b["cnt"][:],
                )
                nc.vector.tensor_scalar(
                    b["cond"][:], b["cnt"][:], b["tgt"][:], None, alu.is_lt
                )
                nc.vector.tensor_scalar(
                    b["loL"][:], b["cond"][:], float(2**i), b["loL"][:],
                    alu.mult, alu.add,
                )
        def compose_tau(t):
            # tau = bitcast((loH << 16) | loL)
            b = bst[t]
            thr16 = b["thr"][:].bitcast(u16).rearrange("p (n two) -> p n two", two=2)
            nc.vector.tensor_copy(thr16[:, :, 1], b["loH"][:])
            nc.vector.tensor_copy(thr16[:, :, 0], b["loL"][:])

        # ---- compact: mask -> ranks -> scatter coords/indices ----
        pcc, grow32, grow_f = [None, None], [None, None], [None, None]
        sel16s = [None, None]

        def compact(t):
            b = bst[t]
            mask = pool.tile([P, NPTS], f32, tag=f"mask{t}", name=f"mask{t}")
            nc.vector.tensor_scalar(mask[:], v[t][:], b["thr"][:], None, alu.is_le)
            rank = pool.tile([P, NPTS], f32, tag=f"rank{t}", name=f"rank{t}")
            nc.vector.tensor_tensor_scan(
                rank[:], mask[:], mask[:], 0.0, alu.add, alu.bypass
            )
            am = pool.tile([P, NPTS], f32, tag=f"am{t}", name=f"am{t}")
            nc.vector.tensor_tensor(am[:], rank[:], mask[:], alu.mult)

            # u16-half slot indices: even half -> 2a-2, odd half -> 2a-1
            idx2 = pool.tile([P, 2 * NPTS], i16, tag=f"idx2_{t}", name=f"idx2_{t}")
            i2v = idx2[:].rearrange("p (n two) -> p n two", two=2)
            nc.scalar.activation(i2v[:, :, 0], am[:], actf.Identity, bias=cm2[:], scale=2.0)
            nc.scalar.activation(i2v[:, :, 1], am[:], actf.Identity, bias=cm1[:], scale=2.0)
            slot16 = pool.tile([P, NPTS], i16, tag=f"slot16{t}", name=f"slot16{t}")
            nc.scalar.activation(slot16[:], am[:], actf.Identity, bias=cm1[:], scale=1.0)

            # compacted coord channels (as u16 halves of f32)
            chc = []
            for c in range(3):
                cc = pool.tile([P, 2 * KPOS], u16, tag=f"cc{c}_{t}", name=f"cc{c}_{t}")
                nc.gpsimd.local_scatter(
                    cc[:], pch[t][c][:].bitcast(u16), idx2[:],
                    channels=P, num_elems=2 * KPOS, num_idxs=2 * NPTS,
                )
                chc.append(cc)
            pcc[t] = [cc[:].bitcast(f32) for cc in chc]

            # compacted original indices (i16 scatter of iota, then widen)
            sel16 = pool.tile([P, KPOS], i16, tag=f"sel16{t}", name=f"sel16{t}")
            nc.gpsimd.local_scatter(
                sel16[:], iota16[:], slot16[:],
                channels=P, num_elems=KPOS, num_idxs=NPTS,
            )
            sel16s[t] = sel16

        def compact_finish(t):
            # row-index conversion + DRAM scratch write: gates only the c4
            # gathers of the FPS loop, so it runs after both tiles' searches
            sel16 = sel16s[t]
            self_f = pool.tile([P, KPOS], f32, tag=f"selff{t}", name=f"selff{t}")
            nc.vector.tensor_copy(self_f[:], sel16[:])
            growf = pool.tile([P, KPOS], f32, tag=f"growf{t}", name=f"growf{t}")
            nc.vector.tensor_scalar(growf[:], self_f[:], xrowf[t][:], None, alu.add)
            g32 = pool.tile([P, KPOS], i32, tag=f"g32_{t}", name=f"g32_{t}")
            nc.vector.tensor_copy(g32[:], growf[:])
            grow32[t] = g32
            grow_f[t] = growf

            # interleave compacted rows [-x,-y,-z,rowbits] -> DRAM
            pci = pool.tile([P, KPOS * 4], f32, tag=f"pil{t}", name=f"pci{t}")
            pciv = pci[:].rearrange("p (n c) -> p n c", c=4)
            for c in range(3):
                nc.vector.tensor_scalar_mul(pciv[:, :, c], pcc[t][c], -1.0)
            nc.vector.tensor_copy(pciv[:, :, 3], g32[:].bitcast(f32))
            nc.sync.dma_start(
                out=pc_dram[t][:].rearrange("(p n) c -> p (n c)", p=P),
                in_=pci[:],
            )

        for t in range(TILES):
            stage_a(t)
            vlm_setup(t)
            stage_b(t)
            compose_tau(t)
            compact(t)
        for t in range(TILES):
            compact_finish(t)

        # ---- FPS state ----
        st = []
        for t in range(TILES):
            s = {}
            s["px"], s["py"], s["pz"] = pcc[t]
            s["m8"] = pool.tile([P, 8], f32, tag=f"m8_{t}", name=f"m8_{t}")
            s["pidx"] = pool.tile([P, 8], u32, tag=f"pidx_{t}", name=f"pidx_{t}")
            s["goffi"] = pool.tile([P, 1], i32, tag=f"goffi_{t}", name=f"goffi_{t}")
            s["c4"] = [
                pool.tile([P, 4], f32, tag=f"c4a_{t}", name=f"c4a_{t}"),
                pool.tile([P, 4], f32, tag=f"c4b_{t}", name=f"c4b_{t}"),
            ]
            s["grows"] = pool.tile([P, K], i32, tag=f"grows_{t}", name=f"grows_{t}")
            s["xg"] = pool.tile([P, K * 32], f32, tag=f"xg_{t}", name=f"xg_{t}")
            s["sqx"] = pool.tile([P, KPOS], f32, tag=f"mask{t}", name=f"fsqx{t}")
            s["sqy"] = pool.tile([P, KPOS], f32, tag=f"rank{t}", name=f"fsqy{t}")
            s["sqz"] = pool.tile([P, KPOS], f32, tag=f"am{t}", name=f"fsqz{t}")
            s["s1"] = pool.tile([P, KPOS], f32, tag=f"ch0_{t}", name=f"fs1{t}")
            s["d2"] = pool.tile([P, KPOS], f32, tag=f"ch1_{t}", name=f"fd2{t}")
            s["md"] = [
                pool.tile([P, KPOS], f32, tag=f"ch2_{t}", name=f"mdA_{t}"),
                pool.tile([P, KPOS], f32, tag=f"idx2_{t}", name=f"mdB_{t}"),
            ]
            s["cneg"] = pool.tile([P, 3], f32, tag=f"cneg_{t}", name=f"cneg_{t}")
            st.append(s)

        def squares(t, bias_ap):
            # d2 terms: exact (p - c)^2 via ACT Square(p + (-c)); fp32 FMA
            # inside ACT keeps the subtraction exactly rounded.
            s = st[t]
            nc.scalar.activation(
                s["sqx"][:], s["px"], actf.Square, bias=bias_ap[:, 0:1], scale=1.0
            )
            nc.scalar.activation(
                s["sqy"][:], s["py"], actf.Square, bias=bias_ap[:, 1:2], scale=1.0
            )
            nc.scalar.activation(
                s["sqz"][:], s["pz"], actf.Square, bias=bias_ap[:, 2:3], scale=1.0
            )

        def d2min(t, dst, src):
            # (sqx + sqy) + sqz, then min with current min_d (same order as ref)
            s = st[t]
            nc.gpsimd.tensor_tensor(s["s1"][:], s["sqx"][:], s["sqy"][:], alu.add)
            nc.gpsimd.tensor_tensor(s["d2"][:], s["s1"][:], s["sqz"][:], alu.add)
            nc.vector.tensor_tensor(dst[:], src[:], s["d2"][:], alu.min)

        def record_x(t, k, c4):
            # store pick's global x-row (bits)
            s = st[t]
            nc.vector.tensor_copy(s["grows"][:, k : k + 1], c4[:, 3:4].bitcast(i32))

        def fetch_x(t, k):
            # gather the pick's feature row (emitted late: keeps the Pool
            # queue clear for the chain-critical c4 dispatches)
            s = st[t]
            nc.gpsimd.indirect_dma_start(
                out=s["xg"][:, k * 32 : (k + 1) * 32],
                out_offset=None,
                in_=x_in[:],
                in_offset=IndirectOffsetOnAxis(ap=s["grows"][:, k : k + 1], axis=0),
            )

        # ---- FPS init: start = argmin v over all 1024 points ----
        for t in range(TILES):
            s = st[t]
            b = bst[t]
            vneg = pool.tile([P, NPTS], f32, tag=f"sqy{t}", name=f"vneg{t}")
            nc.vector.tensor_scalar_mul(vneg[:], v[t][:], -1.0)
            nc.vector.max(s["m8"][:], vneg[:])
            nc.vector.max_index(s["pidx"][:], s["m8"][:], vneg[:])
            # global row of the start pick
            nc.vector.tensor_scalar(
                s["grows"][:, 0:1], s["pidx"][:, 0:1].bitcast(i32), xrowf[t][:],
                None, alu.add,
            )
            nc.gpsimd.indirect_dma_start(
                out=s["c4"][0][:, 0:3],
                out_offset=None,
                in_=pos_in[:],
                in_offset=IndirectOffsetOnAxis(ap=s["grows"][:, 0:1], axis=0),
            )
            nc.gpsimd.indirect_dma_start(
                out=s["xg"][:, 0:32],
                out_offset=None,
                in_=x_in[:],
                in_offset=IndirectOffsetOnAxis(ap=s["grows"][:, 0:1], axis=0),
            )
            nc.vector.tensor_scalar_mul(s["cneg"][:], s["c4"][0][:, 0:3], -1.0)
            squares(t, s["cneg"])
        for t in range(TILES):
            s = st[t]
            nc.gpsimd.tensor_tensor(s["s1"][:], s["sqx"][:], s["sqy"][:], alu.add)
            nc.gpsimd.tensor_tensor(s["md"][0][:], s["s1"][:], s["sqz"][:], alu.add)

        # ---- MLP pieces (emitted in chunks between FPS iterations) ----
        xg3 = [st[t]["xg"][:].rearrange("p (k f) -> p k f", f=32) for t in range(TILES)]
        mlp = []
        with tc.tile_pool(name="psum", bufs=1, space="PSUM") as psp:
            # PSUM tiles are shared between the two tiles (bank budget); the
            # Tile dep tracker serializes their MLP chunks, which is fine —
            # the tensor engine is serial anyway.
            ps_xt = psp.tile([P, 1024], f32, tag="psxt")
            ps_h = psp.tile([64, 512], f32, tag="psh")
            ps_h2 = psp.tile([32, 512], f32, tag="psh2")
            ps_z = psp.tile([4, 512], f32, tag="psz")
            ezc = pool.tile([4, 2 * 1024], f32, tag="ezc", name="ezc")
            s4c = pool.tile([4, 2 * 1024], f32, tag="pil1", name="s4c")
            for t in range(TILES):
                m = {}
                m["ps_xt"] = ps_xt
                m["xt4"] = pool.tile([P, 1024], f32, tag=f"xt4_{t}", name=f"xt4_{t}")
                m["ps_h"] = ps_h
                m["h1"] = pool.tile([64, 512], f32, tag=f"h1_{t}", name=f"h1_{t}")
                m["ps_h2"] = ps_h2
                m["h2"] = pool.tile([32, 512], f32, tag=f"h2_{t}", name=f"h2_{t}")
                m["ps_z"] = ps_z
                m["z4"] = pool.tile([4, 1024], f32, tag=f"z4_{t}", name=f"z4_{t}")
                m["ez"] = ezc[:, t * 1024 : (t + 1) * 1024]
                m["s4"] = s4c[:, t * 1024 : (t + 1) * 1024]
                mlp.append(m)

            def mlp_chunk(t, j0, j1):
                # transpose picks j0..j1-1 into ps_xt, then MLP those columns
                m, s = mlp[t], st[t]
                for j in range(j0, j1):
                    lane, grp = j % 4, j // 4
                    nc.tensor.matmul(
                        m["ps_xt"][lane * 32 : (lane + 1) * 32,
                                   grp * 128 : (grp + 1) * 128],
                        xg3[t][:, j, :],
                        eye[:],
                        tile_position=(0, lane * 32),
                    )
                c0, c1 = (j0 // 4) * 128, (j1 // 4) * 128
                w = c1 - c0
                nc.vector.tensor_copy(m["xt4"][:, c0:c1], m["ps_xt"][:, c0:c1])
                nc.tensor.matmul(m["ps_h"][:, 0:w], w1d[:], m["xt4"][:, c0:c1])
                nc.vector.tensor_scalar(
                    m["h1"][:, 0:w], m["ps_h"][:, 0:w], b1d[:], 0.0, alu.add, alu.max
                )
                nc.tensor.matmul(m["ps_h2"][:, 0:w], w2d[:], m["h1"][:, 0:w])
                nc.vector.tensor_scalar(
                    m["h2"][:, 0:w], m["ps_h2"][:, 0:w], b2d[:], 0.0, alu.add, alu.max
                )
                nc.tensor.matmul(m["ps_z"][:, 0:w], w3d[:], m["h2"][:, 0:w])
                nc.vector.tensor_copy(m["z4"][:, c0:c1], m["ps_z"][:, 0:w])
                # softplus(z + b3) = ln(1 + exp(z + b3)). Only the Exp runs
                # per chunk (its table set also serves Square/Identity); all
                # Ln ops are deferred to the tail so the activation table
                # switches exactly once instead of ping-ponging.
                nc.scalar.activation(
                    m["ez"][:, c0:c1], m["z4"][:, c0:c1], actf.Exp,
                    bias=b3d[:], scale=1.0,
                )
                nc.vector.tensor_scalar(
                    m["ez"][:, c0:c1], m["ez"][:, c0:c1], 1.0, None, alu.add
                )
                del c0, c1

            # ---- FPS loop: two tiles software-pipelined, A leads B ----
            A, B = st[0], st[1]
            for t in range(TILES):
                st[t]["zero"] = pool.tile([P, 1], f32, tag=f"zero_{t}", name=f"zero_{t}")

            def argmax_dispatch(s, t, k, md_cur, couple=None):
                nc.vector.max(s["m8"][:], md_cur[:])
                nc.vector.max_index(s["pidx"][:], s["m8"][:], md_cur[:])
                if couple is None:
                    nc.vector.tensor_scalar(
                        s["goffi"][:], s["pidx"][:, 0:1].bitcast(i32), rb512f[:],
                        None, alu.add,
                    )
                else:
                    # phase separation: a zero produced from the partner
                    # tile's in-flight gather delays this dispatch until the
                    # partner's center data has landed, keeping the two
                    # chains' ACT bursts from colliding.
                    nc.vector.tensor_scalar_mul(s["zero"][:], couple[:, 0:1], 0.0)
                    nc.vector.tensor_scalar(
                        s["goffi"][:], s["pidx"][:, 0:1].bitcast(i32), rb512f[:],
                        s["zero"][:], alu.add, alu.add,
                    )
                nc.gpsimd.indirect_dma_start(
                    out=s["c4"][k % 2][:],
                    out_offset=None,
                    in_=pc_dram[t][:],
                    in_offset=IndirectOffsetOnAxis(ap=s["goffi"][:], axis=0),
                )

            def update(s, t, k):
                # c4 rows hold negated coords: biases directly usable.
                # The last pick needs no min_d update at all - only its row.
                c4 = s["c4"][k % 2]
                if k < K - 1:
                    squares(t, c4)
                    d2min(t, s["md"][k % 2], s["md"][(k - 1) % 2])
                record_x(t, k, c4)

            def argmax_last(s, t):
                # final pick: no min_d update and no center gather - only the
                # pick's global x-row, extracted on-chip from growf via a
                # one-hot reduce (saves a DRAM round trip on the tail chain)
                k = K - 1
                nc.vector.max(s["m8"][:], s["md"][(k - 1) % 2][:])
                nc.vector.max_index(s["pidx"][:], s["m8"][:], s["md"][(k - 1) % 2][:])
                pf = s["zero"]
                nc.vector.tensor_copy(pf[:], s["pidx"][:, 0:1])
                oh = s["s1"]
                nc.vector.tensor_scalar(oh[:], io512f[:], pf[:], None, alu.is_equal)
                mg = s["d2"]
                nc.gpsimd.tensor_tensor(mg[:], grow_f[t][:], oh[:], alu.mult)
                gl = s["goffi"]
                glf = s["zero"]
                nc.vector.tensor_reduce(glf[:], mg[:], mybir.AxisListType.X, alu.add)
                nc.vector.tensor_copy(gl[:], glf[:])
                nc.vector.tensor_copy(s["grows"][:, k : k + 1], gl[:])
                fetch_x(t, k)

            for k in range(1, K - 1):
                argmax_dispatch(A, 0, k, A["md"][(k - 1) % 2])
                if k > 1:
                    update(B, 1, k - 1)
                argmax_dispatch(B, 1, k, B["md"][(k - 1) % 2])
                update(A, 0, k)
                if k > 1:
                    fetch_x(1, k - 1)
                fetch_x(0, k)
                if k == 17:
                    mlp_chunk(0, 0, 16)
                elif k == 18:
                    mlp_chunk(1, 0, 16)
                elif k == 25:
                    mlp_chunk(0, 16, 24)
                elif k == 26:
                    mlp_chunk(1, 16, 24)
                elif k == 29:
                    mlp_chunk(0, 24, 28)
                elif k == 30:
                    mlp_chunk(1, 24, 28)
            argmax_last(A, 0)
            update(B, 1, K - 2)
            fetch_x(1, K - 2)
            argmax_last(B, 1)

            # final MLP chunks (picks 28-31 only)
            mlp_chunk(0, 28, 32)
            mlp_chunk(1, 28, 32)

            # ---- tail ----
            # one fused Ln over both tiles' (1 + exp(z + b3)) columns: a
            # single instruction cannot be interleaved with Exp ops by the
            # scheduler, so the activation table switches exactly once.
            nc.scalar.activation(s4c[:], ezc[:], actf.Ln)
            for t in range(TILES):
                s, m = st[t], mlp[t]
                # indices: local = global - row base
                loc = pool.tile([P, K], i32, tag=f"loc_{t}", name=f"loc_{t}")
                nxr = pool.tile([P, 1], f32, tag=f"nxr_{t}", name=f"nxr_{t}")
                nc.gpsimd.tensor_scalar_mul(nxr[:], xrowf[t][:], -1.0)
                nc.vector.tensor_scalar(
                    loc[:], s["grows"][:], nxr[:], None, alu.add
                )
                nc.sync.dma_start(out=i_out[t * P : (t + 1) * P, :], in_=loc[:])

                ps_w = psp.tile([P, K], f32, tag=f"psW{t}")
                for c in range(8):
                    nc.tensor.transpose(
                        ps_w[:, c * 4 : (c + 1) * 4],
                        m["s4"][:, c * 128 : (c + 1) * 128],
                        eye[0:4, 0:4],
                    )
                wout = pool.tile([P, K], f32, tag=f"wout_{t}", name=f"wout_{t}")
                nc.vector.tensor_copy(wout[:], ps_w[:])
                nc.sync.dma_start(out=w_out[t * P : (t + 1) * P, :], in_=wout[:])

    nc.compile()
    return nc


def _host_prep(W1, b1, W2, b2, W3, b3):
    """Block-diagonal 4-lane weight stacks + replicated biases."""
    W1 = np.asarray(W1, np.float32)
    W2 = np.asarray(W2, np.float32)
    W3 = np.asarray(W3, np.float32)
    W1d = np.zeros((128, 64), np.float32)
    W2d = np.zeros((64, 32), np.float32)
    W3d = np.zeros((32, 4), np.float32)
    for l in range(4):
        W1d[l * 32 : (l + 1) * 32, l * 16 : (l + 1) * 16] = W1
        W2d[l * 16 : (l + 1) * 16, l * 8 : (l + 1) * 8] = W2
        W3d[l * 8 : (l + 1) * 8, l : l + 1] = W3
    b1d = np.tile(np.asarray(b1, np.float32), 4).reshape(64, 1)
    b2d = np.tile(np.asarray(b2, np.float32), 4).reshape(32, 1)
    b3d = np.tile(np.asarray(b3, np.float32), 4).reshape(4, 1)
    return W1d, W2d, W3d, b1d, b2d, b3d


_NC = None


def _get_nc():
    global _NC
    if _NC is None:
        _NC = build_nc()
    return _NC


def kernel(x, pos, batch, W1, b1, W2, b2, W3, b3):
    from concourse.bass_utils import run_bass_kernel_spmd

    x = np.ascontiguousarray(np.asarray(x, np.float32))
    pos = np.ascontiguousarray(np.asarray(pos, np.float32))
    W1d, W2d, W3d, b1d, b2d, b3d = _host_prep(W1, b1, W2, b2, W3, b3)
    eye128 = np.eye(128, dtype=np.float32)

    rows = S_CORE * NPTS
    in_maps = []
    for c in range(N_CORES):
        in_maps.append(
            {
                "pos": pos[c * rows : (c + 1) * rows],
                "x": x[c * rows : (c + 1) * rows],
                "W1d": W1d, "W2d": W2d, "W3d": W3d,
                "b1d": b1d, "b2d": b2d, "b3d": b3d,
                "eye128": eye128,
            }
        )

    nc = _get_nc()
    res = run_bass_kernel_spmd(nc, in_maps, list(range(N_CORES))).results
    weights = np.concatenate([res[c]["weights_out"] for c in range(N_CORES)], axis=0)
    indices = np.concatenate(
        [res[c]["indices_out"].astype(np.int32) for c in range(N_CORES)], axis=0
    )
    return weights, indices
